# revision 4
# baseline (speedup 1.0000x reference)
"""Trainium2 Bass kernel for nn_FKRM_85839216378385 (vq_codebook).

Strategy (8 NeuronCores, SPMD):
  - Attention branch ([B*HW, n_embed] softmax-attention over an 8192-entry
    codebook) is sharded over PIXELS: core i handles image rows
    [12*i, 12*i+12) of both batches = 2304 pixels, attending over the full
    codebook. The softmax is fused (never materialized in HBM): scores^T are
    built codebook-major ([128 codes x pix] tiles) with 4 row-group-packed
    K=3 matmuls running concurrently in the PE array, exp'd on the scalar
    engine straight out of PSUM, and contracted with v_ext = [v | 1] so the
    softmax numerator and denominator come out of one PSUM accumulation.
  - The PSF image-fusion branch needs global per-batch min/max of the cosine
    map, so it is replicated on every core (it is tiny); its contribution to
    the fused output conv (channels 3..5 of fuse_w) is emitted as a separate
    full-image output `ff_full`, and the attention contribution (channels
    0..2 + bias) as a per-core slice `out_a`. The host adds them.
  - All PSF square roots run as fast-inverse-sqrt + 2 Newton steps on the
    vector engine, so the scalar engine's activation table never leaves the
    natural_log_exp set during the hot exp loop (layernorm rstd uses
    exp(-0.5*ln(v+eps)) from the same set).
  - Weight-only transforms (k = k_w @ bg_embed etc.) are repacked on host.
"""

import numpy as np

N_CORES = 8
B, C, H, W = 2, 3, 96, 96
D = 3
NE = 8192
NWIN = 7
PAD = NWIN // 2          # 3
WP = W + 2 * PAD         # 102
HSL = H // N_CORES       # 12 rows per core (per batch)
PIX = B * HSL * W        # 2304 pixels per core
BCW = B * C * W          # 576
NVAR = float(NWIN * NWIN)          # 49
SCALE = float(D) ** -0.5
PCS = [(0, 512), (512, 512), (1024, 512), (1536, 512), (2048, 256)]
NT = NE // 128           # 64 codebook tiles of 128
NJ = NT // 2             # 32 pairs of row-packed tiles


def _build_program(reps=1):
    import sys
    if "/opt/trn_rl_repo" not in sys.path:
        sys.path.insert(0, "/opt/trn_rl_repo")
    import concourse.bass as bass
    import concourse.mybir as mybir
    import concourse.tile as tile
    from concourse import bacc
    import concourse.bass_isa as bass_isa
    from contextlib import ExitStack

    f32 = mybir.dt.float32
    f32r = mybir.dt.float32r
    u32 = mybir.dt.uint32
    AF = mybir.ActivationFunctionType
    ALU = mybir.AluOpType
    AX = mybir.AxisListType
    ROP = bass_isa.ReduceOp

    nc = bacc.Bacc("TRN2", target_bir_lowering=False, debug=False,
                   num_devices=N_CORES)

    # ---------------- dram I/O ----------------
    d_front = nc.dram_tensor("front", [B, C, H, W], f32, kind="ExternalInput")
    d_back = nc.dram_tensor("back", [B, C, H, W], f32, kind="ExternalInput")
    d_xcm = nc.dram_tensor("front_cm", [D, PIX], f32, kind="ExternalInput")
    d_k4 = nc.dram_tensor("k4", [12, NT // 4 * 128], f32r, kind="ExternalInput")
    d_v = nc.dram_tensor("v_sb", [128, NT * 4], f32r, kind="ExternalInput")
    d_band = nc.dram_tensor("band", [H, H], f32, kind="ExternalInput")
    d_w1T = nc.dram_tensor("w1T", [3, 6], f32, kind="ExternalInput")
    d_b1 = nc.dram_tensor("b1", [6], f32, kind="ExternalInput")
    d_w2cT = nc.dram_tensor("w2cT", [6, 3], f32, kind="ExternalInput")
    d_b2c = nc.dram_tensor("b2c", [3], f32, kind="ExternalInput")
    d_qwT = nc.dram_tensor("qwT", [3, 3], f32, kind="ExternalInput")
    d_n1g = nc.dram_tensor("n1g", [3], f32, kind="ExternalInput")
    d_n1b = nc.dram_tensor("n1b", [3], f32, kind="ExternalInput")
    d_mo1T = nc.dram_tensor("mo1T", [3, 6], f32, kind="ExternalInput")
    d_mob1 = nc.dram_tensor("mob1", [6], f32, kind="ExternalInput")
    d_mo2cT = nc.dram_tensor("mo2cT", [6, 3], f32, kind="ExternalInput")
    d_mob2c = nc.dram_tensor("mob2c", [3], f32, kind="ExternalInput")
    d_n2g = nc.dram_tensor("n2g", [3], f32, kind="ExternalInput")
    d_n2b = nc.dram_tensor("n2b", [3], f32, kind="ExternalInput")
    d_fuseA = nc.dram_tensor("fuseT_a", [4, 3], f32, kind="ExternalInput")
    d_cw = nc.dram_tensor("cw", [45], f32, kind="ExternalInput")
    d_cbias = nc.dram_tensor("cbias", [4 * BCW], f32, kind="ExternalInput")
    d_eps3 = nc.dram_tensor("eps3", [3], f32, kind="ExternalInput")
    d_ones33 = nc.dram_tensor("ones33", [3, 3], f32, kind="ExternalInput")

    d_oa = nc.dram_tensor("out_a", [D, PIX], f32, kind="ExternalOutput")
    d_ff = nc.dram_tensor("ff_full", [B, C, H, W], f32, kind="ExternalOutput")

    def hbcw_ap(handle, b):
        """AP over one batch of a [B,C,H,W] dram tensor ordered (h | c, w)."""
        a = handle[:, :, :, :]
        return bass.AP(tensor=a.tensor, offset=a.offset + b * C * H * W,
                       ap=[[W, H], [H * W, C], [1, W]])

    def col_ap(handle, n):
        """[n] dram vector viewed as [n, 1] (one element per partition)."""
        a = handle[:]
        return bass.AP(tensor=a.tensor, offset=a.offset, ap=[[1, n], [0, 1]])

    def bcast_ap(handle, n):
        """[n] dram vector broadcast across 96 partitions -> [96, n]."""
        a = handle[:]
        return bass.AP(tensor=a.tensor, offset=a.offset, ap=[[0, 96], [1, n]])

    with tile.TileContext(nc) as tc, ExitStack() as ctx:
        consts = ctx.enter_context(tc.tile_pool(name="consts", bufs=1))
        psf = ctx.enter_context(tc.tile_pool(name="psf", bufs=1))
        psft = ctx.enter_context(tc.tile_pool(name="psft", bufs=2))
        mlp = ctx.enter_context(tc.tile_pool(name="mlp", bufs=1))
        attn = ctx.enter_context(tc.tile_pool(name="attn", bufs=3))

        # ---------------- constants to SBUF ----------------
        k4_sb = consts.tile([99, NT // 4 * 128], f32r)
        k_eng = [nc.sync, nc.scalar, nc.gpsimd, nc.sync]
        for g in range(4):
            k_eng[g].dma_start(out=k4_sb[32 * g:32 * g + 3, :],
                               in_=d_k4[3 * g:3 * g + 3, :])
        v_sb = consts.tile([128, NT * 4], f32r)
        nc.sync.dma_start(out=v_sb[:, 0:NT * 2], in_=d_v[:, 0:NT * 2])
        nc.scalar.dma_start(out=v_sb[:, NT * 2:], in_=d_v[:, NT * 2:])
        v_sb4 = v_sb.rearrange("p (n f) -> p n f", f=4)
        band_sb = consts.tile([H, H], f32)
        nc.sync.dma_start(out=band_sb, in_=d_band[:, :])
        w1T_sb = consts.tile([3, 6], f32)
        nc.sync.dma_start(out=w1T_sb, in_=d_w1T[:, :])
        w2cT_sb = consts.tile([6, 3], f32)
        nc.sync.dma_start(out=w2cT_sb, in_=d_w2cT[:, :])
        qwT_sb = consts.tile([3, 3], f32)
        nc.sync.dma_start(out=qwT_sb, in_=d_qwT[:, :])
        mo1T_sb = consts.tile([3, 6], f32)
        nc.sync.dma_start(out=mo1T_sb, in_=d_mo1T[:, :])
        mo2cT_sb = consts.tile([6, 3], f32)
        nc.sync.dma_start(out=mo2cT_sb, in_=d_mo2cT[:, :])
        fuseA_sb = consts.tile([4, 3], f32)
        nc.sync.dma_start(out=fuseA_sb, in_=d_fuseA[:, :])
        ones33_sb = consts.tile([3, 3], f32)
        nc.sync.dma_start(out=ones33_sb, in_=d_ones33[:, :])
        b1_sb = consts.tile([6, 1], f32)
        nc.sync.dma_start(out=b1_sb, in_=col_ap(d_b1, 6))
        b2c_sb = consts.tile([3, 1], f32)
        nc.sync.dma_start(out=b2c_sb, in_=col_ap(d_b2c, 3))
        n1g_sb = consts.tile([3, 1], f32)
        nc.sync.dma_start(out=n1g_sb, in_=col_ap(d_n1g, 3))
        n1b_sb = consts.tile([3, 1], f32)
        nc.sync.dma_start(out=n1b_sb, in_=col_ap(d_n1b, 3))
        mob1_sb = consts.tile([6, 1], f32)
        nc.sync.dma_start(out=mob1_sb, in_=col_ap(d_mob1, 6))
        mob2c_sb = consts.tile([3, 1], f32)
        nc.sync.dma_start(out=mob2c_sb, in_=col_ap(d_mob2c, 3))
        n2g_sb = consts.tile([3, 1], f32)
        nc.sync.dma_start(out=n2g_sb, in_=col_ap(d_n2g, 3))
        n2b_sb = consts.tile([3, 1], f32)
        nc.sync.dma_start(out=n2b_sb, in_=col_ap(d_n2b, 3))
        eps3_sb = consts.tile([3, 1], f32)
        nc.sync.dma_start(out=eps3_sb, in_=col_ap(d_eps3, 3))
        cw_sb = consts.tile([96, 45], f32)
        nc.gpsimd.dma_start(out=cw_sb, in_=bcast_ap(d_cw, 45))
        cbias_sb = consts.tile([96, 4 * BCW], f32)
        nc.gpsimd.dma_start(out=cbias_sb, in_=bcast_ap(d_cbias, 4 * BCW))

        def gelu_exp(dst, x_ps, bias_sb, nparts, tA, tB, tC, tD, tE):
            """dst = gelu_tanh(x_ps + bias) via exp (stays in the nle
            activation-table set): gelu(x) = x * e/(1+e),
            e = exp(2*0.7978845608*(x + 0.044715*x^3))."""
            xg = mlp.tile([nparts, PIX], f32, tag=tA)
            nc.vector.tensor_scalar(xg, x_ps, bias_sb, None, ALU.add)
            t = mlp.tile([nparts, PIX], f32, tag=tB)
            nc.gpsimd.tensor_mul(t, xg, xg)
            nc.gpsimd.tensor_mul(t, t, xg)
            nc.gpsimd.tensor_scalar_mul(t, t, 0.044715)
            nc.gpsimd.tensor_add(t, t, xg)
            e = mlp.tile([nparts, PIX], f32, tag=tC)
            nc.scalar.activation(e, t, AF.Exp, scale=1.5957691216057308)
            d = mlp.tile([nparts, PIX], f32, tag=tD)
            nc.gpsimd.tensor_scalar_add(d, e, 1.0)
            r = mlp.tile([nparts, PIX], f32, tag=tE)
            nc.vector.reciprocal(r, d)
            nc.vector.tensor_mul(r, e, r)
            nc.vector.tensor_mul(dst, xg, r)

        for _rep in range(reps):
            # =========================================================
            # PSF branch (full images, replicated on every core)
            # layout L0: [h=96 partitions | (b, c, w)]
            # =========================================================
            fr = psf.tile([96, B, C, WP], f32, tag="ps11")
            bk = psf.tile([96, B, C, WP], f32)
            nc.vector.memset(fr, 0.0)
            nc.vector.memset(bk, 0.0)
            eng_cycle = [nc.sync, nc.scalar, nc.gpsimd, nc.sync]
            for b in range(B):
                eng_cycle[b].dma_start(out=fr[:, b, :, PAD:PAD + W],
                                       in_=hbcw_ap(d_front, b))
                eng_cycle[2 + b].dma_start(out=bk[:, b, :, PAD:PAD + W],
                                           in_=hbcw_ap(d_back, b))
            sqf = psf.tile([96, B, C, WP], f32, tag="ps1")
            sqb = psf.tile([96, B, C, WP], f32, tag="ps2")
            nc.gpsimd.tensor_mul(sqf, fr, fr)
            nc.gpsimd.tensor_mul(sqb, bk, bk)

            def hbox(dst, src):
                """dst[96,B,C,W] = sum_j src[96,B,C,j:j+W] (7-tap along w)."""
                nc.vector.tensor_add(dst, src[:, :, :, 0:W], src[:, :, :, 1:1 + W])
                for j in range(2, NWIN):
                    nc.vector.tensor_add(dst, dst, src[:, :, :, j:j + W])

            brh_f = psf.tile([96, B, C, W], f32, tag="ps3")
            brh_f2 = psf.tile([96, B, C, W], f32, tag="ps4")
            brh_b = psf.tile([96, B, C, W], f32, tag="ps5")
            brh_b2 = psf.tile([96, B, C, W], f32, tag="ps6")
            hbox(brh_f, fr)
            hbox(brh_f2, sqf)
            hbox(brh_b, bk)
            hbox(brh_b2, sqb)

            s1f = psf.tile([96, B, C, W], f32, tag="ps7")
            s2f = psf.tile([96, B, C, W], f32, tag="ps8")
            s1b = psf.tile([96, B, C, W], f32, tag="ps9")
            s2b = psf.tile([96, B, C, W], f32, tag="ps10")

            with tc.tile_pool(name="ps_vbox", bufs=1, space="PSUM") as ps_vbox, \
                 tc.tile_pool(name="ps_big_a", bufs=1, space="PSUM") as psA:

                def vbox(dst, src):
                    """dst = Band^T @ src over the h (partition) axis."""
                    sflat = src.rearrange("p b c w -> p (b c w)")
                    dflat = dst.rearrange("p b c w -> p (b c w)")
                    for half in range(2):
                        sl = slice(half * 288, half * 288 + 288)
                        bp = ps_vbox.tile([96, 288], f32, tag="vbox_ps", bufs=1)
                        nc.tensor.matmul(bp, band_sb, sflat[:, sl],
                                         start=True, stop=True)
                        nc.vector.tensor_copy(dflat[:, sl], bp)

                vbox(s1f, brh_f)
                vbox(s2f, brh_f2)
                vbox(s1b, brh_b)
                vbox(s2b, brh_b2)

                # ---- mlp_in + ln1 + q (channel-major [d | pix]) ----
                X_sb = mlp.tile([D, PIX], f32, tag="slotA")
                nc.sync.dma_start(out=X_sb, in_=d_xcm[:, :])
                h_sb = mlp.tile([6, PIX], f32, tag="slotB")
                y_sb = mlp.tile([3, PIX], f32, tag="slotE")
                sq_sb = mlp.tile([3, PIX], f32, tag="slotC")
                l_sb = mlp.tile([3, PIX], f32, tag="slotF")
                r3_sb = mlp.tile([3, PIX], f32, tag="slotG")
                x2_sb = mlp.tile([3, PIX], f32, tag="slotB2")
                MCS = [(i * 512, min(512, PIX - i * 512)) for i in range(5)]

                # chunked head pipeline: the attention loop's first pixel
                # chunk only needs q4[:, 0:512], so run the whole chain per
                # 512-pixel chunk to start the exp loop early.
                q4_sb = mlp.tile([99, PIX], f32r, tag="slotQ")
                for off, n in MCS:
                    sl = slice(off, off + n)
                    h_ps = psA.tile([6, 512], f32, tag="hps", bufs=2)
                    nc.tensor.matmul(h_ps[:, 0:n], w1T_sb, X_sb[:, sl],
                                     start=True, stop=True)
                    nc.scalar.activation(h_sb[:, sl], h_ps[:, 0:n],
                                         AF.Gelu_apprx_tanh, bias=b1_sb)
                    y_ps = psA.tile([3, 512], f32, tag="yps", bufs=2)
                    nc.tensor.matmul(y_ps[:, 0:n], w2cT_sb, h_sb[:, sl],
                                     start=True, stop=True)
                    nc.vector.tensor_scalar(y_sb[:, sl], y_ps[:, 0:n],
                                            b2c_sb, None, ALU.add)
                    nc.vector.tensor_mul(sq_sb[:, sl], y_sb[:, sl], y_sb[:, sl])
                    v_ps = psA.tile([3, 512], f32, tag="vps", bufs=1)
                    nc.tensor.matmul(v_ps[:, 0:n], ones33_sb, sq_sb[:, sl],
                                     start=True, stop=True)
                    nc.scalar.activation(l_sb[:, sl], v_ps[:, 0:n],
                                         AF.Ln, bias=eps3_sb)
                    nc.scalar.activation(r3_sb[:, sl], l_sb[:, sl],
                                         AF.Exp, scale=-0.5)
                    nc.vector.tensor_mul(x2_sb[:, sl], y_sb[:, sl], r3_sb[:, sl])
                    nc.vector.tensor_scalar(x2_sb[:, sl], x2_sb[:, sl],
                                            n1g_sb, n1b_sb, ALU.mult, ALU.add)
                    q_ps = psA.tile([3, 512], f32, tag="qps", bufs=1)
                    nc.tensor.matmul(q_ps[:, 0:n], qwT_sb, x2_sb[:, sl],
                                     start=True, stop=True)
                    # replicate q at partition groups {0,32,64,96} (row packing)
                    for g in range(4):
                        nc.vector.tensor_copy(q4_sb[32 * g:32 * g + 3, sl],
                                              q_ps[:, 0:n])

            # ---- PSF stats (overlap the attention loop on DVE) ----
            def stats(s1, s2, mean_t, rstd_t, sd_t):
                """mean = s1/49 ; v = (s2 - s1^2/49)/48 ; sd = sqrt(v);
                rstd = 1/(sd + 1e-8)."""
                v_t = psft.tile([96, B, C, W], f32, tag="st_v")
                u_t = psft.tile([96, B, C, W], f32, tag="st_u")
                rs_t = psft.tile([96, B, C, W], f32, tag="st_r")
                nc.vector.tensor_mul(v_t, s1, s1)
                nc.vector.tensor_scalar_mul(v_t, v_t, -1.0 / (NVAR * (NVAR - 1)))
                nc.vector.tensor_scalar_mul(u_t, s2, 1.0 / (NVAR - 1))
                nc.vector.tensor_add(v_t, v_t, u_t)
                # sqrt via exp(0.5*ln(v)) -- stays in the natural_log_exp set
                nc.scalar.activation(rs_t, v_t, AF.Ln)
                nc.scalar.activation(sd_t, rs_t, AF.Exp, scale=0.5)
                nc.vector.tensor_scalar_add(u_t, sd_t, 1e-8)
                nc.vector.reciprocal(rstd_t, u_t)
                nc.vector.tensor_scalar_mul(mean_t, s1, 1.0 / NVAR)

            m_f = psf.tile([96, B, C, W], f32, tag="ps1")
            r_f = psf.tile([96, B, C, W], f32, tag="ps2")
            sd_f = psf.tile([96, B, C, W], f32, tag="ps12")
            stats(s1f, s2f, m_f, r_f, sd_f)
            m_b = psf.tile([96, B, C, W], f32, tag="ps3")
            r_b = psf.tile([96, B, C, W], f32, tag="ps4")
            sd_b = psf.tile([96, B, C, W], f32, tag="ps5")
            stats(s1b, s2b, m_b, r_b, sd_b)

            # mvnorm(front), mvnorm(back), adain
            xnf = psf.tile([96, B, C, W], f32, tag="ps6")
            nc.vector.tensor_sub(xnf, fr[:, :, :, PAD:PAD + W], m_f)
            nc.vector.tensor_mul(xnf, xnf, r_f)
            xnb = psf.tile([96, B, C, W], f32, tag="ps7")
            nc.vector.tensor_sub(xnb, bk[:, :, :, PAD:PAD + W], m_b)
            nc.vector.tensor_mul(xnb, xnb, r_b)
            xad = psf.tile([96, B, C, W], f32, tag="ps8")
            nc.vector.tensor_mul(xad, xnf, sd_b)
            nc.vector.tensor_add(xad, xad, m_b)

            def conv3(dst, src, wbase, bias_idx=None):
                """1x1 conv over c: dst[:,b,co,w] = sum_ci w[co,ci]*src[:,b,ci,w]."""
                for co in range(3):
                    dco = dst[:, :, co, :]
                    t = psft.tile([96, B, W], f32, tag="conv_t")
                    nc.vector.tensor_scalar_mul(
                        dco, src[:, :, 0, :],
                        cw_sb[:, wbase + co * 3:wbase + co * 3 + 1])
                    nc.vector.tensor_scalar_mul(
                        t, src[:, :, 1, :],
                        cw_sb[:, wbase + co * 3 + 1:wbase + co * 3 + 2])
                    nc.vector.tensor_add(dco, dco, t)
                    nc.vector.tensor_scalar_mul(
                        t, src[:, :, 2, :],
                        cw_sb[:, wbase + co * 3 + 2:wbase + co * 3 + 3])
                    nc.vector.tensor_add(dco, dco, t)
                if bias_idx is not None:
                    dflat = dst.rearrange("p b c w -> p (b c w)")
                    nc.vector.tensor_add(
                        dflat, dflat,
                        cbias_sb[:, bias_idx * BCW:(bias_idx + 1) * BCW])

            EE = psf.tile([96, B, C, W], f32, tag="ps9")
            FF = psf.tile([96, B, C, W], f32, tag="ps13")
            GG = psf.tile([96, B, C, W], f32, tag="ps11")
            HH = psf.tile([96, B, C, W], f32, tag="ps14")
            conv3(EE, xad, 0, bias_idx=0)
            conv3(FF, xnf, 9, bias_idx=1)
            conv3(GG, xnb, 18, bias_idx=2)
            conv3(HH, bk[:, :, :, PAD:PAD + W], 27, bias_idx=3)

            # cosine similarity S[h, b, w] = dot * rsqrt(F2*G2)
            dot = psf.tile([96, B, W], f32)
            f2 = psf.tile([96, B, W], f32)
            g2 = psf.tile([96, B, W], f32)
            tmc = psft.tile([96, B, W], f32, tag="cos_t")
            nc.vector.tensor_mul(dot, FF[:, :, 0, :], GG[:, :, 0, :])
            nc.vector.tensor_mul(f2, FF[:, :, 0, :], FF[:, :, 0, :])
            nc.vector.tensor_mul(g2, GG[:, :, 0, :], GG[:, :, 0, :])
            for cc in range(1, 3):
                nc.vector.tensor_mul(tmc, FF[:, :, cc, :], GG[:, :, cc, :])
                nc.vector.tensor_add(dot, dot, tmc)
                nc.vector.tensor_mul(tmc, FF[:, :, cc, :], FF[:, :, cc, :])
                nc.vector.tensor_add(f2, f2, tmc)
                nc.vector.tensor_mul(tmc, GG[:, :, cc, :], GG[:, :, cc, :])
                nc.vector.tensor_add(g2, g2, tmc)
            nc.vector.tensor_mul(f2, f2, g2)          # F2*G2
            rs2 = psf.tile([96, B, W], f32)
            nc.scalar.activation(g2, f2, AF.Ln)
            nc.scalar.activation(f2, g2, AF.Exp, scale=-0.5)   # 1/(Fn*Gn)
            S = psf.tile([96, B, W], f32)
            nc.vector.tensor_mul(S, dot, f2)

            # global min/max per batch: free-dim reduce then gpsimd all-reduce
            # over partitions (min via max of negated values)
            rmx = psf.tile([96, 2], f32)
            rmn_neg = psf.tile([96, 2], f32)
            nS = psf.tile([96, B, W], f32)
            nc.vector.tensor_scalar_mul(nS, S, -1.0)
            nc.vector.tensor_reduce(rmx, S, axis=AX.X, op=ALU.max)
            nc.vector.tensor_reduce(rmn_neg, nS, axis=AX.X, op=ALU.max)
            mx_bc = psf.tile([96, 2], f32)
            nmn_bc = psf.tile([96, 2], f32)
            nc.gpsimd.partition_all_reduce(mx_bc, rmx, 96, ROP.max)
            nc.gpsimd.partition_all_reduce(nmn_bc, rmn_neg, 96, ROP.max)
            dd_bc = psf.tile([96, 2], f32)
            nc.vector.tensor_add(dd_bc, mx_bc, nmn_bc)    # max - min
            ri_bc = psf.tile([96, 2], f32)
            nc.vector.reciprocal(ri_bc, dd_bc)
            Sn = psf.tile([96, B, W], f32)
            for b in range(B):
                # (S + (-min)) * (1/(max-min))
                nc.vector.tensor_scalar(
                    Sn[:, b, :], S[:, b, :],
                    nmn_bc[:, b:b + 1], ri_bc[:, b:b + 1],
                    ALU.add, ALU.mult)

            # fused = HH + Sn*(EE-HH)
            fused = psf.tile([96, B, C, W], f32, tag="ps10")
            nc.vector.tensor_sub(fused, EE, HH)
            for cc in range(3):
                nc.vector.tensor_mul(fused[:, :, cc, :], fused[:, :, cc, :], Sn)
            ff_flat = fused.rearrange("p b c w -> p (b c w)")
            hh_flat = HH.rearrange("p b c w -> p (b c w)")
            nc.vector.tensor_add(ff_flat, ff_flat, hh_flat)

            # ff contribution: conv with fuse_w[:, 3:6] (no bias)
            ffo = psf.tile([96, B, C, W], f32, tag="ps13")
            conv3(ffo, fused, 36, bias_idx=None)
            for b in range(B):
                nc.sync.dma_start(out=hbcw_ap(d_ff, b), in_=ffo[:, b, :, :])

            # =========================================================
            # attention main loop  (codebook-major score tiles,
            # 4 row-group-packed score matmuls per PSUM tile)
            # =========================================================
            att_sb = mlp.tile([3, PIX], f32, tag="slotD")
            z3_sb = attn.tile([3, 512], f32, tag="z3", bufs=1)
            nc.vector.memset(z3_sb, 0.0)
            with tc.tile_pool(name="ps_sc", bufs=3, space="PSUM") as ps_sc, \
                 tc.tile_pool(name="ps_num", bufs=1, space="PSUM") as ps_num:
                for off, n in PCS:
                    num_ps = ps_num.tile([4, 512], f32, tag="num")
                    for j in range(NJ):
                        r0 = 2 * (j % 2)   # alternate row-group pairs per j
                        m = j // 2
                        sc_ps = ps_sc.tile([128, 1024], f32, tag="sc")
                        for g in range(2):
                            r = r0 + g
                            nc.tensor.matmul(
                                sc_ps[:, g * 512:g * 512 + n],
                                k4_sb[32 * r:32 * r + 3, m * 128:(m + 1) * 128],
                                q4_sb[32 * r:32 * r + 3, off:off + n],
                                tile_position=(32 * r, 0),
                                start=True, stop=True)
                        ex_t = attn.tile([128, 1024], f32r, tag="ex")
                        sc_view = sc_ps.rearrange("p (g c) -> p g c", g=2)[:, :, 0:n]
                        nc.scalar.activation(ex_t[:, 0:2 * n], sc_view, AF.Exp)
                        for g in range(2):
                            nt = 2 * j + g
                            nc.tensor.matmul(
                                num_ps[:, 0:n], v_sb4[:, nt, :],
                                ex_t[:, g * n:(g + 1) * n],
                                start=(j == 0 and g == 0),
                                stop=(j == NJ - 1 and g == 1))
                    # epilogue: att = num[0:3] / num[3]
                    num_sb = attn.tile([4, 512], f32, tag="numsb", bufs=2)
                    nc.vector.tensor_copy(num_sb[:, 0:n], num_ps[:, 0:n])
                    den_sb = attn.tile([1, 512], f32, tag="den", bufs=2)
                    nc.sync.dma_start(out=den_sb[:, 0:n], in_=num_sb[3:4, 0:n])
                    nc.vector.reciprocal(z3_sb[0:1, 0:n], den_sb[:, 0:n])
                    r3a_sb = attn.tile([3, 512], f32, tag="r3a", bufs=2)
                    nc.gpsimd.partition_all_reduce(r3a_sb[:, 0:n],
                                                   z3_sb[:, 0:n], 3, ROP.add)
                    nc.vector.tensor_mul(att_sb[:, off:off + n],
                                         num_sb[0:3, 0:n], r3a_sb[:, 0:n])

            # =========================================================
            # mlp_out + ln2 + fused output conv (attention part)
            # =========================================================
            h2_sb = mlp.tile([6, PIX], f32, tag="slotB")
            y2_sb = mlp.tile([3, PIX], f32, tag="slotE")
            sq2_sb = mlp.tile([3, PIX], f32, tag="slotC")
            l2_sb = mlp.tile([3, PIX], f32, tag="slotF")
            r32_sb = mlp.tile([3, PIX], f32, tag="slotG")
            x4_sb = mlp.tile([4, PIX], f32, tag="slotA")
            oa_sb = mlp.tile([3, PIX], f32, tag="slotB2")
            MCS = [(i * 512, min(512, PIX - i * 512)) for i in range(5)]
            with tc.tile_pool(name="ps_big_b", bufs=1, space="PSUM") as psB:
                h2_ps = psB.tile([6, PIX], f32, tag="bigb")
                for off, n in MCS:
                    nc.tensor.matmul(h2_ps[:, off:off + n], mo1T_sb,
                                     att_sb[:, off:off + n], start=True, stop=True)
                nc.scalar.activation(h2_sb, h2_ps, AF.Gelu_apprx_tanh, bias=mob1_sb)

                y2_ps = psB.tile([3, PIX], f32, tag="bigb")
                for off, n in MCS:
                    nc.tensor.matmul(y2_ps[:, off:off + n], mo2cT_sb,
                                     h2_sb[:, off:off + n], start=True, stop=True)
                nc.vector.tensor_scalar(y2_sb, y2_ps, mob2c_sb, None, ALU.add)
                nc.vector.tensor_mul(sq2_sb, y2_sb, y2_sb)

                v2_ps = psB.tile([3, PIX], f32, tag="bigb")
                for off, n in MCS:
                    nc.tensor.matmul(v2_ps[:, off:off + n], ones33_sb,
                                     sq2_sb[:, off:off + n], start=True, stop=True)
                nc.scalar.activation(l2_sb, v2_ps, AF.Ln, bias=eps3_sb)
                nc.scalar.activation(r32_sb, l2_sb, AF.Exp, scale=-0.5)
                nc.vector.memset(x4_sb, 1.0)   # row 3 stays 1 (fuse bias input)
                nc.vector.tensor_mul(x4_sb[0:3, :], y2_sb, r32_sb)
                nc.vector.tensor_scalar(x4_sb[0:3, :], x4_sb[0:3, :],
                                        n2g_sb, n2b_sb, ALU.mult, ALU.add)

                o_ps = psB.tile([3, PIX], f32, tag="bigb")
                for off, n in MCS:
                    nc.tensor.matmul(o_ps[:, off:off + n], fuseA_sb,
                                     x4_sb[:, off:off + n], start=True, stop=True)
                nc.vector.tensor_copy(oa_sb, o_ps)
            nc.sync.dma_start(out=d_oa[:, :], in_=oa_sb)

    nc.compile()
    return nc


_CACHED = {}


def _prepare_in_maps(inputs):
    f = lambda k: np.asarray(inputs[k], np.float32)
    front, back = f("front"), f("back")
    bg = f("bg_embed")                      # [3, 8192]
    q_w, k_w, v_w = f("q_w"), f("k_w"), f("v_w")
    mi_w1, mi_b1 = f("mi_w1"), f("mi_b1")
    mi_w2, mi_b2 = f("mi_w2"), f("mi_b2")
    mo_w1, mo_b1 = f("mo_w1"), f("mo_b1")
    mo_w2, mo_b2 = f("mo_w2"), f("mo_b2")
    n1_g, n1_b, n2_g, n2_b = f("n1_g"), f("n1_b"), f("n2_g"), f("n2_b")
    e_w, e_b = f("e_w"), f("e_b")
    f_w, f_b = f("f_w"), f("f_b")
    g_w, g_b = f("g_w"), f("g_b")
    h_w, h_b = f("h_w"), f("h_b")
    fuse_w, fuse_b = f("fuse_w"), f("fuse_b")

    # ---- host-side weight repacking (tiny, O(n_embed * d)) ----
    kT = (k_w @ bg) * SCALE                                   # [3, NE]
    # row-group-packed k: pair (j, g) -> row group r = 2*(j%2)+g, col m = j//2
    k4 = np.zeros((12, NT // 4 * 128), np.float32)
    for j in range(NJ):
        for g in range(2):
            r = 2 * (j % 2) + g
            m = j // 2
            nt = 2 * j + g
            k4[3 * r:3 * r + 3, m * 128:(m + 1) * 128] = \
                kT[:, nt * 128:(nt + 1) * 128]
    v = bg.T @ v_w.T                                          # [NE, 3]
    v_ext = np.concatenate([v, np.ones((NE, 1), np.float32)], 1)
    v_np = np.ascontiguousarray(
        v_ext.reshape(NT, 128, 4).transpose(1, 0, 2).reshape(128, NT * 4))
    hh, ww = np.meshgrid(np.arange(H), np.arange(H), indexing="ij")
    band = (np.abs(hh - ww) <= PAD).astype(np.float32)
    w2c = mi_w2 - mi_w2.mean(0, keepdims=True)
    b2c = mi_b2 - mi_b2.mean()
    mo2c = mo_w2 - mo_w2.mean(0, keepdims=True)
    mob2c = mo_b2 - mo_b2.mean()
    fuseT_a = np.concatenate([fuse_w[:, 0:3].T, fuse_b[None, :]], 0)
    cw = np.concatenate([e_w.ravel(), f_w.ravel(), g_w.ravel(),
                         h_w.ravel(), fuse_w[:, 3:6].ravel()])
    cbias = np.concatenate(
        [np.tile(np.repeat(bb, W), B) for bb in (e_b, f_b, g_b, h_b)])

    common = dict(
        front=front, back=back,
        k4=k4, v_sb=v_np,
        band=band,
        w1T=np.ascontiguousarray(mi_w1.T), b1=mi_b1,
        w2cT=np.ascontiguousarray(w2c.T), b2c=b2c,
        qwT=np.ascontiguousarray(q_w.T),
        n1g=n1_g, n1b=n1_b,
        mo1T=np.ascontiguousarray(mo_w1.T), mob1=mo_b1,
        mo2cT=np.ascontiguousarray(mo2c.T), mob2c=mob2c,
        n2g=n2_g, n2b=n2_b,
        fuseT_a=np.ascontiguousarray(fuseT_a),
        cw=np.ascontiguousarray(cw, np.float32),
        cbias=np.ascontiguousarray(cbias, np.float32),
        eps3=np.full(3, 1e-5, np.float32),
        ones33=np.full((3, 3), 1.0 / 3.0, np.float32),
    )
    common = {k: np.ascontiguousarray(v2, np.float32)
              for k, v2 in common.items()}

    in_maps = []
    for i in range(N_CORES):
        sl = front[:, :, HSL * i:HSL * (i + 1), :]          # [B,3,12,96]
        xcm = np.ascontiguousarray(
            sl.transpose(1, 0, 2, 3).reshape(D, PIX), np.float32)
        in_maps.append(dict(common, front_cm=xcm))
    return in_maps


def _gather_output(res):
    out = np.array(res.results[0]["ff_full"], np.float32)
    for i in range(N_CORES):
        oa = res.results[i]["out_a"].reshape(D, B, HSL, W)
        out[:, :, HSL * i:HSL * (i + 1), :] += oa.transpose(1, 0, 2, 3)
    return out


def kernel(**inputs):
    import sys
    if "/opt/trn_rl_repo" not in sys.path:
        sys.path.insert(0, "/opt/trn_rl_repo")
    from concourse.bass_utils import run_bass_kernel_spmd

    in_maps = _prepare_in_maps(inputs)
    if "nc" not in _CACHED:
        _CACHED["nc"] = _build_program()
    nc = _CACHED["nc"]

    res = run_bass_kernel_spmd(nc, in_maps, core_ids=list(range(N_CORES)))
    return _gather_output(res)



# revision 49
# speedup vs baseline: 373.2562x; 373.2562x over previous
"""Trainium2 Bass kernel for nn_FKRM_85839216378385 (vq_codebook).

Strategy (8 NeuronCores, SPMD):
  - Attention branch ([B*HW, n_embed] softmax-attention over an 8192-entry
    codebook) is sharded over PIXELS: core i handles image rows
    [12*i, 12*i+12) of both batches = 2304 pixels, attending over the full
    codebook. The softmax is fused (never materialized in HBM): scores^T are
    built codebook-major ([128 codes x pix] tiles, 3 tiles per 3-bank PSUM
    buffer) with row-group-packed K=3 matmuls, exp'd on the scalar engine
    straight out of PSUM in 1536-wide calls, and contracted with
    v_ext = [v | 1] so the softmax numerator and denominator come out of one
    PSUM accumulation.
  - The program is software-pipelined around the Activation engine (the
    bottleneck: ~123us of exp work per core at 0.83ns/col): the PSF
    image-fusion branch (replicated; needs global per-batch min/max) and the
    per-chunk mlp_out tails are emitted as small filler steps between score
    tile groups so they run on DVE/Pool/PE while Activation streams exps.
  - Only Exp/Ln/Gelu activation functions are used, ordered so the table
    loads happen exactly twice (gelu set once at the head, natural_log_exp
    for everything after).
  - Weight-only transforms (k = k_w @ bg_embed etc.) are repacked on host.
"""

import numpy as np

N_CORES = 8
B, C, H, W = 2, 3, 96, 96
D = 3
NE = 8192
NWIN = 7
PAD = NWIN // 2          # 3
WP = W + 2 * PAD         # 102
HSL = H // N_CORES       # 12 rows per core (per batch)
PIX = B * HSL * W        # 2304 pixels per core
BCW = B * C * W          # 576
NVAR = float(NWIN * NWIN)          # 49
SCALE = float(D) ** -0.5
PCS = [(0, 256), (256, 512), (768, 512), (1280, 512), (1792, 512)]
NT = NE // 128           # 64 codebook tiles of 128
NRG = 3                  # k row groups (PE col-tile dst must be 0/32/64)
NKB = (NT + NRG - 1) // NRG   # 22 column blocks in the packed k
GRP = 3                  # score tiles per PSUM buffer (3 banks)


def _build_program(reps=1):
    import sys
    if "/opt/trn_rl_repo" not in sys.path:
        sys.path.insert(0, "/opt/trn_rl_repo")
    import concourse.bass as bass
    import concourse.mybir as mybir
    import concourse.tile as tile
    from concourse import bacc
    import concourse.bass_isa as bass_isa
    from contextlib import ExitStack

    f32 = mybir.dt.float32
    f32r = mybir.dt.float32r
    u32 = mybir.dt.uint32
    AF = mybir.ActivationFunctionType
    ALU = mybir.AluOpType
    AX = mybir.AxisListType
    ROP = bass_isa.ReduceOp

    nc = bacc.Bacc("TRN2", target_bir_lowering=False, debug=False,
                   num_devices=N_CORES)

    # ---------------- dram I/O ----------------
    d_front = nc.dram_tensor("front", [B, C, H, W], f32, kind="ExternalInput")
    d_back = nc.dram_tensor("back", [B, C, H, W], f32, kind="ExternalInput")
    d_xcm = nc.dram_tensor("front_cm", [D, PIX], f32r, kind="ExternalInput")
    d_k4 = nc.dram_tensor("k4", [3 * NRG, NKB * 128], f32r,
                          kind="ExternalInput")
    d_v = nc.dram_tensor("v_sb", [128, NT * 4], f32r, kind="ExternalInput")
    d_band = nc.dram_tensor("band", [H, H], f32r, kind="ExternalInput")
    # all small weight matrices packed into one [6, 30] tensor (one DMA):
    # [0:6,0:3]=w2cT [0:6,3:6]=mo2cT [0:3,6:12]=w1T [0:3,12:15]=qwT
    # [0:3,15:21]=mo1T [0:3,21:24]=fuse3T [0:3,24:27]=ones33
    # [0:1,27:30]=ones13 [0:4,30:33]=sel43 (row-3 selector)
    d_wpack = nc.dram_tensor("wpack", [6, 33], f32r, kind="ExternalInput")
    # all bias/scale vectors packed into one [6, 10] tensor (one DMA):
    # cols: b1, mob1, b2c, n1g, n1b, mob2c, n2g, n2b, fuseb, eps3
    d_vpack = nc.dram_tensor("vpack", [6, 10], f32, kind="ExternalInput")
    d_cw = nc.dram_tensor("cw", [45], f32, kind="ExternalInput")
    d_cbias = nc.dram_tensor("cbias", [4 * BCW], f32, kind="ExternalInput")

    d_oa = nc.dram_tensor("out_a", [D, PIX], f32, kind="ExternalOutput")
    d_ff = nc.dram_tensor("ff_full", [B, C, H, W], f32, kind="ExternalOutput")

    def hbcw_ap(handle, b):
        """AP over one batch of a [B,C,H,W] dram tensor ordered (h | c, w)."""
        a = handle[:, :, :, :]
        return bass.AP(tensor=a.tensor, offset=a.offset + b * C * H * W,
                       ap=[[W, H], [H * W, C], [1, W]])

    def col_ap(handle, n):
        """[n] dram vector viewed as [n, 1] (one element per partition)."""
        a = handle[:]
        return bass.AP(tensor=a.tensor, offset=a.offset, ap=[[1, n], [0, 1]])

    def bcast_ap(handle, n):
        """[n] dram vector broadcast across 96 partitions -> [96, n]."""
        a = handle[:]
        return bass.AP(tensor=a.tensor, offset=a.offset, ap=[[0, 96], [1, n]])

    with tile.TileContext(nc) as tc, ExitStack() as ctx:
        consts = ctx.enter_context(tc.tile_pool(name="consts", bufs=1))
        psf = ctx.enter_context(tc.tile_pool(name="psf", bufs=1))
        psft = ctx.enter_context(tc.tile_pool(name="psft", bufs=2))
        mlp = ctx.enter_context(tc.tile_pool(name="mlp", bufs=1))
        attn = ctx.enter_context(tc.tile_pool(name="attn", bufs=3))
        tl = ctx.enter_context(tc.tile_pool(name="tl", bufs=2))

        # ---------------- constants to SBUF ----------------
        # head-critical loads (X, wpack, vpack) lead the SP queue; the big
        # loop constants (k4, v) go on the gpsimd queue so nothing blocks
        # the mlp_in head pipeline.
        X_sb = mlp.tile([D, PIX], f32r, tag="X")
        nc.sync.dma_start(out=X_sb, in_=d_xcm[:, :])
        wp_sb = consts.tile([6, 33], f32r)
        nc.sync.dma_start(out=wp_sb, in_=d_wpack[:, :])
        vp_sb = consts.tile([6, 10], f32)
        nc.sync.dma_start(out=vp_sb, in_=d_vpack[:, :])
        w2cT_sb = wp_sb[0:6, 0:3]
        mo2cT_sb = wp_sb[0:6, 3:6]
        w1T_sb = wp_sb[0:3, 6:12]
        qwT_sb = wp_sb[0:3, 12:15]
        mo1T_sb = wp_sb[0:3, 15:21]
        fuse3_sb = wp_sb[0:3, 21:24]
        ones33_sb = wp_sb[0:3, 24:27]
        ones13_sb = wp_sb[0:1, 27:30]
        sel43_sb = wp_sb[0:4, 30:33]
        b1_sb = vp_sb[0:6, 0:1]
        mob1_sb = vp_sb[0:6, 1:2]
        b2c_sb = vp_sb[0:3, 2:3]
        n1g_sb = vp_sb[0:3, 3:4]
        n1b_sb = vp_sb[0:3, 4:5]
        mob2c_sb = vp_sb[0:3, 5:6]
        n2g_sb = vp_sb[0:3, 6:7]
        n2b_sb = vp_sb[0:3, 7:8]
        fuseb_sb = vp_sb[0:3, 8:9]
        eps3_sb = vp_sb[0:3, 9:10]
        k4_sb = consts.tile([67, NKB * 128], f32r)
        for g in range(NRG):
            nc.gpsimd.dma_start(out=k4_sb[32 * g:32 * g + 3, :],
                                in_=d_k4[3 * g:3 * g + 3, :])
        v_sb = consts.tile([128, NT * 4], f32r)
        nc.gpsimd.dma_start(out=v_sb[:, 0:NT * 2], in_=d_v[:, 0:NT * 2])
        nc.gpsimd.dma_start(out=v_sb[:, NT * 2:], in_=d_v[:, NT * 2:])
        v_sb4 = v_sb.rearrange("p (n f) -> p n f", f=4)
        band_sb = consts.tile([H, H], f32r)
        nc.gpsimd.dma_start(out=band_sb, in_=d_band[:, :])
        cw_sb = consts.tile([96, 45], f32)
        nc.gpsimd.dma_start(out=cw_sb, in_=bcast_ap(d_cw, 45))
        cbias_sb = consts.tile([96, 4 * BCW], f32)
        nc.gpsimd.dma_start(out=cbias_sb, in_=bcast_ap(d_cbias, 4 * BCW))
        # fast-inverse-sqrt magic constant, replicated for tensor_sub
        magic_sb = consts.tile([96, BCW], u32)
        nc.vector.memset(magic_sb, 0x5F3759DF)

        def emit_rsqrt(eng, dst, src, mk_tmp, newton=1):
            """dst = 1/sqrt(src) via the u32 bit trick + Newton steps.

            All ops run on `eng` (DVE or gpsimd). src/dst are f32 APs with
            identical [p, n] shape, n <= BCW, p <= 96, contiguous innermost.
            mk_tmp() yields scratch f32 APs of the same shape.
            """
            p, n = src.partition_size(), src.free_size()
            y = dst
            yu = y.bitcast(u32)
            # scalar-operand ops only lower on DVE (Pool fails the codegen
            # engine check for TensorScalar); plain muls may run on `eng`
            nc.vector.tensor_scalar(yu, src.bitcast(u32), 1, None,
                                    ALU.logical_shift_right)
            eng.tensor_sub(yu, magic_sb[0:p, 0:n], yu)
            for _ in range(newton):
                t = mk_tmp()
                eng.tensor_mul(t, y, y)
                eng.tensor_mul(t, t, src)
                nc.vector.tensor_scalar(t, t, -0.5, 1.5, ALU.mult, ALU.add)
                eng.tensor_mul(y, y, t)

        for _rep in range(reps):
            # =========================================================
            # PSF input staging (independent of everything else)
            # layout L0: [h=96 partitions | (b, c, w)]
            # =========================================================
            fr = psf.tile([96, B, C, WP], f32, tag="fr")
            bk = psf.tile([96, B, C, WP], f32, tag="bk")
            for t in (fr, bk):
                nc.vector.memset(t[:, :, :, 0:PAD], 0.0)
                nc.vector.memset(t[:, :, :, PAD + W:], 0.0)
            fb_eng = [nc.gpsimd, nc.gpsimd, nc.sync, nc.sync]
            for b in range(B):
                fb_eng[b].dma_start(out=fr[:, b, :, PAD:PAD + W],
                                    in_=hbcw_ap(d_front, b))
                fb_eng[2 + b].dma_start(out=bk[:, b, :, PAD:PAD + W],
                                        in_=hbcw_ap(d_back, b))

            # =========================================================
            # head: mlp_in + ln1 + q, chunked (channel-major [d | pix]).
            # All 5 gelus run back-to-back on Act (one gelu table load);
            # the rest of the head uses no Act at all (DVE bit-trick rsqrt
            # for the ln1 rstd). Only chunk 0 completes before the
            # attention loop; chunks 1-4 are emitted as in-loop fillers.
            # =========================================================
            q4_sb = mlp.tile([67, PIX], f32r, tag="q4")
            hsb = {}

            def make_head_steps(c, alloc):
                """Three filler steps computing q4 for chunk c."""
                off, n = PCS[c]
                st = {}

                def mk_tmp():
                    return tl.tile([6, 512], f32, tag="t_tmp", bufs=3,
                                   name=f"nt_{c}")[0:3, 0:n]

                def hd1():
                    yp = alloc(3, 512, f"yps_{c}")
                    nc.tensor.matmul(yp[:, 0:n], w2cT_sb, hsb[c][:, 0:n],
                                     start=True, stop=True)
                    y_t = tl.tile([3, 512], f32, tag="y", name=f"y_{c}")
                    nc.vector.tensor_scalar(y_t[:, 0:n], yp[:, 0:n],
                                            b2c_sb, None, ALU.add)
                    sq_t = tl.tile([6, 512], f32r, tag="t_tmp", bufs=3,
                                   name=f"sq_{c}")[0:3, :]
                    nc.vector.tensor_mul(sq_t[:, 0:n], y_t[:, 0:n],
                                         y_t[:, 0:n])
                    st["y"], st["sq"] = y_t, sq_t

                def hd2():
                    vp = alloc(3, 512, f"vps_{c}")
                    nc.tensor.matmul(vp[:, 0:n], ones33_sb,
                                     st["sq"][:, 0:n], start=True, stop=True)
                    u_t = tl.tile([6, 512], f32, tag="t_tmp", bufs=3,
                                  name=f"u_{c}")[0:3, :]
                    nc.vector.tensor_scalar_add(u_t[:, 0:n], vp[:, 0:n], 1e-5)
                    r_t = tl.tile([3, 512], f32, tag="y", name=f"rst_{c}")
                    emit_rsqrt(nc.vector, r_t[:, 0:n], u_t[:, 0:n], mk_tmp,
                               newton=1)
                    st["r"] = r_t

                def hd3():
                    x2_t = tl.tile([6, 512], f32r, tag="t_tmp", bufs=3,
                                   name=f"x2_{c}")[0:3, :]
                    nc.vector.tensor_mul(x2_t[:, 0:n], st["y"][:, 0:n],
                                         st["r"][:, 0:n])
                    nc.vector.tensor_scalar(x2_t[:, 0:n], x2_t[:, 0:n],
                                            n1g_sb, n1b_sb, ALU.mult, ALU.add)
                    qp = alloc(3, 512, f"qps_{c}")
                    nc.tensor.matmul(qp[:, 0:n], qwT_sb, x2_t[:, 0:n],
                                     start=True, stop=True)
                    # replicate q to the three k row groups (PE column
                    # tiling is rejected by codegen; DVE writes to 32-aligned
                    # partition bases are fine)
                    for g in range(NRG):
                        nc.vector.tensor_copy(
                            q4_sb[32 * g:32 * g + 3, off:off + n], qp[:, 0:n])

                return [hd1, hd2, hd3]

            with tc.tile_pool(name="ps_head", bufs=1, space="PSUM") as psH:
                hps = {}

                def emit_gelu(c):
                    off, n = PCS[c]
                    # bufs=5: h chunks 1-4 stay live until their in-loop
                    # y-matmul fillers
                    ht = tl.tile([6, 512], f32r, tag="hsb", bufs=5,
                                 name=f"h_{c}")
                    nc.scalar.activation(ht[:, 0:n], hps[c][:, 0:n],
                                         AF.Gelu_apprx_tanh, bias=b1_sb)
                    hsb[c] = ht

                for c, (off, n) in enumerate(PCS):
                    hp = psH.tile([6, 512], f32, tag="hps", bufs=2,
                                  name=f"hps_{c}")
                    nc.tensor.matmul(hp[:, 0:n], w1T_sb, X_sb[:, off:off + n],
                                     start=True, stop=True)
                    hps[c] = hp
                    if c >= 1:
                        emit_gelu(c - 1)
                emit_gelu(len(PCS) - 1)

                # preload the exp table while the head pipeline runs, so
                # the first attention exp doesn't pay the 1.28us table load
                dume = tl.tile([1, 8], f32, tag="dume", bufs=1)
                nc.scalar.activation(dume[:, 0:1], vp_sb[0:1, 0:1], AF.Exp)

                def psh_alloc(p, w, name):
                    return psH.tile([p, w], f32, tag="hd", bufs=2, name=name)

                for fn in make_head_steps(0, psh_alloc):
                    fn()

            # =========================================================
            # PSF stage A: squares (Pool) + 7-tap horizontal box (DVE)
            # =========================================================
            sqf = psf.tile([96, B, C, WP], f32, tag="sqf")
            sqb = psf.tile([96, B, C, WP], f32, tag="sqb")
            nc.gpsimd.tensor_mul(sqf, fr, fr)
            nc.gpsimd.tensor_mul(sqb, bk, bk)

            def hbox_half(dst, src, first):
                """7-tap horizontal box sum, split in two filler halves."""
                def fn():
                    if first:
                        nc.vector.tensor_add(dst, src[:, :, :, 0:W],
                                             src[:, :, :, 1:1 + W])
                        for j in range(2, 4):
                            nc.vector.tensor_add(dst, dst,
                                                 src[:, :, :, j:j + W])
                    else:
                        for j in range(4, NWIN):
                            nc.vector.tensor_add(dst, dst,
                                                 src[:, :, :, j:j + W])
                return fn

            brh_f = psf.tile([96, B, C, W], f32r, tag="brh_f")
            brh_f2 = psf.tile([96, B, C, W], f32r, tag="brh_f2")
            brh_b = psf.tile([96, B, C, W], f32r, tag="brh_b")
            brh_b2 = psf.tile([96, B, C, W], f32r, tag="brh_b2")

            # PSF result tiles (filled by filler steps inside the loop;
            # vbox overwrites brh_* in place, so s1f==brh_f etc.)
            m_f = psf.tile([96, B, C, W], f32, tag="m_f")
            r_f = psf.tile([96, B, C, W], f32, tag="r_f")
            m_b = psf.tile([96, B, C, W], f32, tag="m_b")
            r_b = psf.tile([96, B, C, W], f32, tag="r_b")
            sd_b = psf.tile([96, B, C, W], f32, tag="sd_b")
            xnf = psf.tile([96, B, C, W], f32, tag="xnf")
            xnb = psf.tile([96, B, C, W], f32, tag="xnb")
            xad = psf.tile([96, B, C, W], f32, tag="xad")
            EE = psf.tile([96, B, C, W], f32, tag="EE")
            FF = psf.tile([96, B, C, W], f32, tag="FF")
            GG = psf.tile([96, B, C, W], f32, tag="GG")
            HH = psf.tile([96, B, C, W], f32, tag="HH")
            dot = psf.tile([96, B, W], f32, tag="dot")
            f2 = psf.tile([96, B, W], f32, tag="f2")
            g2 = psf.tile([96, B, W], f32, tag="g2")
            S = psf.tile([96, B, W], f32, tag="S")
            Sn = psf.tile([96, B, W], f32, tag="Sn")

            # =========================================================
            # attention main loop, with PSF + mlp_out tails as fillers
            # =========================================================
            with tc.tile_pool(name="ps_sc", bufs=2, space="PSUM") as ps_sc, \
                 tc.tile_pool(name="ps_num", bufs=1, space="PSUM") as ps_num, \
                 tc.tile_pool(name="ps_aux", bufs=1, space="PSUM") as ps_aux:

                # ---------- filler step definitions ----------
                def aux_tile(p, w, name):
                    return ps_aux.tile([p, w], f32, tag="aux", name=name)

                def vbox_step(srcdst, idx):
                    """srcdst = Band^T @ srcdst over the h (partition) axis,
                    in place (each 288-col half is matmul'd then written
                    back)."""
                    def fn():
                        sflat = srcdst.rearrange("p b c w -> p (b c w)")
                        for half in range(2):
                            slh = slice(half * 288, half * 288 + 288)
                            bp = aux_tile(96, 288, f"vb_{idx}_{half}")
                            nc.tensor.matmul(bp, band_sb, sflat[:, slh],
                                             start=True, stop=True)
                            nc.vector.tensor_copy(sflat[:, slh], bp)
                    return fn

                def stats1_step(s1, s2, tagp):
                    def fn():
                        v_t = psft.tile([96, B, C, W], f32, tag="st_v",
                                        bufs=1, name=f"v_{tagp}")
                        u_t = psft.tile([96, B, C, W], f32, tag="st_u",
                                        bufs=1, name=f"u_{tagp}")
                        nc.vector.tensor_mul(v_t, s1, s1)
                        nc.vector.tensor_scalar_mul(
                            v_t, v_t, -1.0 / (NVAR * (NVAR - 1)))
                        nc.vector.tensor_scalar_mul(u_t, s2, 1.0 / (NVAR - 1))
                        nc.vector.tensor_add(v_t, v_t, u_t)
                        self_d[tagp] = v_t
                    return fn

                self_d = {}

                def stats2_step(s1, mean_t, rstd_t, sd_t, tagp):
                    def fn():
                        # rstd = rsqrt(var) on DVE (no act-table traffic);
                        # sd (needed only for 'back') = var * rstd
                        v_t = self_d[tagp]
                        vfl = v_t.rearrange("p b c w -> p (b c w)")
                        rfl = rstd_t.rearrange("p b c w -> p (b c w)")

                        def mk_tmp():
                            t = psft.tile([96, B, C, W], f32, tag="st_r",
                                          bufs=1, name=f"rs_{tagp}")
                            return t.rearrange("p b c w -> p (b c w)")

                        emit_rsqrt(nc.vector, rfl, vfl, mk_tmp, newton=1)
                        if sd_t is not None:
                            nc.vector.tensor_mul(sd_t, v_t, rstd_t)
                        nc.vector.tensor_scalar_mul(mean_t, s1, 1.0 / NVAR)
                    return fn

                def mvn_step(dst, src, m_t, r_t):
                    def fn():
                        nc.vector.tensor_sub(dst, src[:, :, :, PAD:PAD + W], m_t)
                        nc.vector.tensor_mul(dst, dst, r_t)
                    return fn

                def adain_step():
                    nc.vector.tensor_mul(xad, xnf, sd_b)
                    nc.vector.tensor_add(xad, xad, m_b)

                def conv3(dst, src, wbase, bias_idx, name):
                    def fn():
                        for co in range(3):
                            dco = dst[:, :, co, :]
                            t = psft.tile([96, B, W], f32, tag="conv_t",
                                          name=f"cv_{name}_{co}")
                            nc.vector.tensor_scalar_mul(
                                dco, src[:, :, 0, :],
                                cw_sb[:, wbase + co * 3:wbase + co * 3 + 1])
                            nc.vector.tensor_scalar_mul(
                                t, src[:, :, 1, :],
                                cw_sb[:, wbase + co * 3 + 1:wbase + co * 3 + 2])
                            nc.vector.tensor_add(dco, dco, t)
                            nc.vector.tensor_scalar_mul(
                                t, src[:, :, 2, :],
                                cw_sb[:, wbase + co * 3 + 2:wbase + co * 3 + 3])
                            nc.vector.tensor_add(dco, dco, t)
                        if bias_idx is not None:
                            dflat = dst.rearrange("p b c w -> p (b c w)")
                            nc.vector.tensor_add(
                                dflat, dflat,
                                cbias_sb[:, bias_idx * BCW:(bias_idx + 1) * BCW])
                    return fn

                def cos1_step():
                    tmc = psft.tile([96, B, W], f32, tag="cos_t", name="tmc1")
                    nc.vector.tensor_mul(dot, FF[:, :, 0, :], GG[:, :, 0, :])
                    nc.vector.tensor_mul(f2, FF[:, :, 0, :], FF[:, :, 0, :])
                    nc.vector.tensor_mul(g2, GG[:, :, 0, :], GG[:, :, 0, :])
                    nc.vector.tensor_mul(tmc, FF[:, :, 1, :], GG[:, :, 1, :])
                    nc.vector.tensor_add(dot, dot, tmc)

                def cos2_step():
                    tmc = psft.tile([96, B, W], f32, tag="cos_t", name="tmc2")
                    nc.vector.tensor_mul(tmc, FF[:, :, 1, :], FF[:, :, 1, :])
                    nc.vector.tensor_add(f2, f2, tmc)
                    nc.vector.tensor_mul(tmc, GG[:, :, 1, :], GG[:, :, 1, :])
                    nc.vector.tensor_add(g2, g2, tmc)
                    nc.vector.tensor_mul(tmc, FF[:, :, 2, :], GG[:, :, 2, :])
                    nc.vector.tensor_add(dot, dot, tmc)

                def cos3_step():
                    tmc = psft.tile([96, B, W], f32, tag="cos_t", name="tmc3")
                    nc.vector.tensor_mul(tmc, FF[:, :, 2, :], FF[:, :, 2, :])
                    nc.vector.tensor_add(f2, f2, tmc)
                    nc.vector.tensor_mul(tmc, GG[:, :, 2, :], GG[:, :, 2, :])
                    nc.vector.tensor_add(g2, g2, tmc)
                    nc.vector.tensor_mul(f2, f2, g2)          # F2*G2
                    # 1/(Fn*Gn) = rsqrt(F2*G2) on DVE
                    f2fl = f2.rearrange("p b w -> p (b w)")
                    g2fl = g2.rearrange("p b w -> p (b w)")

                    def mk_tmp():
                        t = psft.tile([96, B, W], f32, tag="cos_t",
                                      name="tmc4")
                        return t.rearrange("p b w -> p (b w)")

                    emit_rsqrt(nc.vector, g2fl, f2fl, mk_tmp, newton=1)
                    nc.vector.tensor_mul(S, dot, g2)

                mm_d = {}

                def minmax1_step():
                    rmx = psft.tile([96, 2], f32, tag="rmx", name="rmx")
                    rmn = psft.tile([96, 2], f32, tag="rmn", name="rmn")
                    nS = psft.tile([96, B, W], f32, tag="nS", name="nS")
                    nc.vector.tensor_scalar_mul(nS, S, -1.0)
                    nc.vector.tensor_reduce(rmx, S, axis=AX.X, op=ALU.max)
                    nc.vector.tensor_reduce(rmn, nS, axis=AX.X, op=ALU.max)
                    mm_d["rmx"], mm_d["rmn"] = rmx, rmn

                def minmax2_step():
                    nmn_bc = psft.tile([96, 2], f32, tag="nmn_bc", name="nmn_bc")
                    mx_bc = psft.tile([96, 2], f32, tag="mx_bc", name="mx_bc")
                    dd_bc = psft.tile([96, 2], f32, tag="dd_bc", name="dd_bc")
                    ri_bc = psft.tile([96, 2], f32, tag="ri_bc", name="ri_bc")
                    nc.gpsimd.partition_all_reduce(mx_bc, mm_d["rmx"], 96, ROP.max)
                    nc.gpsimd.partition_all_reduce(nmn_bc, mm_d["rmn"], 96, ROP.max)
                    nc.vector.tensor_add(dd_bc, mx_bc, nmn_bc)    # max - min
                    nc.vector.reciprocal(ri_bc, dd_bc)
                    mm_d["nmn"], mm_d["ri"] = nmn_bc, ri_bc

                def minmax3_step():
                    for b in range(B):
                        nc.vector.tensor_scalar(
                            Sn[:, b, :], S[:, b, :],
                            mm_d["nmn"][:, b:b + 1], mm_d["ri"][:, b:b + 1],
                            ALU.add, ALU.mult)

                def fuse1_step():
                    nc.vector.tensor_sub(xnf, EE, HH)   # reuse xnf as scratch
                    for cc in range(3):
                        nc.vector.tensor_mul(xnf[:, :, cc, :],
                                             xnf[:, :, cc, :], Sn)

                def fuse2_step():
                    ff_flat = xnf.rearrange("p b c w -> p (b c w)")
                    hh_flat = HH.rearrange("p b c w -> p (b c w)")
                    nc.vector.tensor_add(ff_flat, ff_flat, hh_flat)

                def ffdma_step():
                    for b in range(B):
                        nc.sync.dma_start(out=hbcw_ap(d_ff, b),
                                          in_=xnb[:, b, :, :])

                # ---------- mlp_out tail steps for one chunk ----------
                # transient [<=6, 512] scratch shares one 3-deep rotating tag
                tail_d = {}

                def tmp_tile(c, nm):
                    return tl.tile([6, 512], f32, tag="t_tmp", bufs=3,
                                   name=f"{nm}_{c}")

                def t0_step(c):
                    def fn():
                        off, n = PCS[c]
                        h2p = aux_tile(6, 512, f"h2p_{c}")
                        nc.tensor.matmul(h2p[:, 0:n], mo1T_sb,
                                         tail_d[c]["att"][:, 0:n],
                                         start=True, stop=True)
                        xg = tl.tile([6, 512], f32, tag="t_xg", name=f"xg_{c}")
                        nc.vector.tensor_scalar(xg[:, 0:n], h2p[:, 0:n],
                                                mob1_sb, None, ALU.add)
                        tail_d[c]["xg"] = xg
                    return fn

                def t1_step(c):
                    def fn():
                        off, n = PCS[c]
                        d = tail_d[c]
                        eng = nc.gpsimd if c < len(PCS) - 1 else nc.vector
                        s = tmp_tile(c, "s")
                        eng.tensor_mul(s[:, 0:n], d["xg"][:, 0:n],
                                       d["xg"][:, 0:n])
                        eng.tensor_mul(s[:, 0:n], s[:, 0:n],
                                       d["xg"][:, 0:n])
                        d["cb"] = s
                    return fn

                def t2_step(c):
                    def fn():
                        off, n = PCS[c]
                        d = tail_d[c]
                        t = tmp_tile(c, "t")
                        nc.vector.tensor_scalar_mul(t[:, 0:n], d["cb"][:, 0:n],
                                                    0.044715)
                        nc.vector.tensor_add(t[:, 0:n], t[:, 0:n],
                                             d["xg"][:, 0:n])
                        e = tmp_tile(c, "e")
                        nc.scalar.activation(e[:, 0:n], t[:, 0:n], AF.Exp,
                                             scale=1.5957691216057308)
                        d["e"] = e
                    return fn

                def t3_step(c):
                    def fn():
                        off, n = PCS[c]
                        d = tail_d[c]
                        dd = tmp_tile(c, "dd")
                        nc.vector.tensor_scalar_add(dd[:, 0:n], d["e"][:, 0:n],
                                                    1.0)
                        r = tmp_tile(c, "r")
                        nc.vector.reciprocal(r[:, 0:n], dd[:, 0:n])
                        nc.vector.tensor_mul(r[:, 0:n], d["e"][:, 0:n],
                                             r[:, 0:n])
                        h2 = tl.tile([6, 512], f32r, tag="t_h2", name=f"h2_{c}")
                        nc.vector.tensor_mul(h2[:, 0:n], d["xg"][:, 0:n],
                                             r[:, 0:n])
                        d["h2"] = h2
                    return fn

                def t4_step(c):
                    def fn():
                        off, n = PCS[c]
                        d = tail_d[c]
                        y2p = aux_tile(3, 512, f"y2p_{c}")
                        nc.tensor.matmul(y2p[:, 0:n], mo2cT_sb,
                                         d["h2"][:, 0:n], start=True, stop=True)
                        y2 = tl.tile([3, 512], f32, tag="t_y2", name=f"ty2_{c}")
                        nc.vector.tensor_scalar(y2[:, 0:n], y2p[:, 0:n],
                                                mob2c_sb, None, ALU.add)
                        sq2 = tl.tile([3, 512], f32r, tag="t_sq2",
                                      name=f"tsq_{c}")
                        nc.vector.tensor_mul(sq2[:, 0:n], y2[:, 0:n],
                                             y2[:, 0:n])
                        d["y2"], d["sq2"] = y2, sq2
                    return fn

                def t5_step(c):
                    # rstd for ln2 = rsqrt(var + eps), off the Act engine
                    # (gpsimd for pipelined chunks, DVE for the last one
                    # where end-to-end latency matters)
                    def fn():
                        off, n = PCS[c]
                        d = tail_d[c]
                        eng = nc.gpsimd if c < len(PCS) - 1 else nc.vector
                        v2p = aux_tile(3, 512, f"v2p_{c}")
                        nc.tensor.matmul(v2p[:, 0:n], ones33_sb,
                                         d["sq2"][:, 0:n], start=True, stop=True)
                        u2 = tmp_tile(c, "u2")
                        # gpsimd cannot read PSUM; the eps-add stays on DVE
                        nc.vector.tensor_scalar_add(u2[0:3, 0:n], v2p[:, 0:n],
                                                    1e-5)
                        r32 = tl.tile([3, 512], f32, tag="t_r32",
                                      name=f"tr32_{c}")

                        def mk_tmp():
                            return tmp_tile(c, "rt")[0:3, 0:n]

                        emit_rsqrt(eng, r32[:, 0:n], u2[0:3, 0:n], mk_tmp,
                                   newton=1)
                        d["r32"] = r32
                    return fn

                def t6_step(c):
                    def fn():
                        off, n = PCS[c]
                        d = tail_d[c]
                        x5 = tl.tile([3, 512], f32r, tag="t_x5", bufs=3,
                                     name=f"x5_{c}")
                        nc.vector.tensor_mul(x5[:, 0:n], d["y2"][:, 0:n],
                                             d["r32"][:, 0:n])
                        nc.vector.tensor_scalar(x5[:, 0:n], x5[:, 0:n],
                                                n2g_sb, n2b_sb,
                                                ALU.mult, ALU.add)
                        d["x5"] = x5
                    return fn

                def t7_step(c):
                    def fn():
                        off, n = PCS[c]
                        d = tail_d[c]
                        op = aux_tile(3, 512, f"op_{c}")
                        nc.tensor.matmul(op[:, 0:n], fuse3_sb, d["x5"][:, 0:n],
                                         start=True, stop=True)
                        oa = tl.tile([3, 512], f32, tag="t_x5", bufs=3,
                                    name=f"oa_{c}")
                        nc.vector.tensor_scalar(oa[:, 0:n], op[:, 0:n],
                                                fuseb_sb, None, ALU.add)
                        nc.sync.dma_start(out=d_oa[:, off:off + n],
                                          in_=oa[:, 0:n])
                    return fn

                def t0g_step(c):
                    # last chunk only: gelu via the act table (Act is idle
                    # once the loop exps end, and the short chain trims the
                    # end-of-program latency)
                    def fn():
                        off, n = PCS[c]
                        h2p = aux_tile(6, 512, f"h2p_{c}")
                        nc.tensor.matmul(h2p[:, 0:n], mo1T_sb,
                                         tail_d[c]["att"][:, 0:n],
                                         start=True, stop=True)
                        h2 = tl.tile([6, 512], f32r, tag="t_h2",
                                     name=f"h2_{c}")
                        nc.scalar.activation(h2[:, 0:n], h2p[:, 0:n],
                                             AF.Gelu_apprx_tanh, bias=mob1_sb)
                        tail_d[c]["h2"] = h2
                    return fn

                def tail_steps(c):
                    if c == len(PCS) - 1:
                        return [t0g_step(c), t4_step(c), t5_step(c),
                                t6_step(c), t7_step(c)]
                    return [t0_step(c), t1_step(c), t2_step(c), t3_step(c),
                            t4_step(c), t5_step(c), t6_step(c), t7_step(c)]

                # ---------- filler schedule: slot (chunk, group) -> fns ----
                slot = {}

                def put(c, gi, fn):
                    slot.setdefault((c, gi), []).append(fn)

                # head chunks 1-4: hd1/hd2 early in chunk c-1; hd3 at
                # gi 9 so its q-matmuls sit well after the rsqrt chain
                for hc in range(1, 5):
                    h1f, h2f, h3f = make_head_steps(hc, aux_tile)
                    put(hc - 1, 1, h1f)
                    put(hc - 1, 3, h2f)
                    put(hc - 1, 9, h3f)

                # mlp_out tails, even slots
                for tc_ in range(4):
                    for i, fn in enumerate(tail_steps(tc_)):
                        put(tc_ + 1, 6 + 2 * i, fn)

                # PSF pipeline, odd slots
                put(0, 11, hbox_half(brh_f, fr, True))
                put(0, 13, hbox_half(brh_f, fr, False))
                put(0, 15, hbox_half(brh_f2, sqf, True))
                put(0, 17, hbox_half(brh_f2, sqf, False))
                put(0, 19, hbox_half(brh_b, bk, True))
                put(0, 21, hbox_half(brh_b, bk, False))
                put(1, 1, hbox_half(brh_b2, sqb, True))
                put(1, 3, hbox_half(brh_b2, sqb, False))
                put(1, 7, vbox_step(brh_f, 0))
                put(1, 9, vbox_step(brh_f2, 1))
                put(1, 11, vbox_step(brh_b, 2))
                put(1, 13, vbox_step(brh_b2, 3))
                put(1, 15, stats1_step(brh_f, brh_f2, "f"))
                put(1, 17, stats2_step(brh_f, m_f, r_f, None, "f"))
                put(1, 19, stats1_step(brh_b, brh_b2, "b"))
                put(1, 21, stats2_step(brh_b, m_b, r_b, sd_b, "b"))
                put(2, 7, mvn_step(xnf, fr, m_f, r_f))
                put(2, 9, mvn_step(xnb, bk, m_b, r_b))
                put(2, 11, adain_step)
                put(2, 13, conv3(EE, xad, 0, 0, "EE"))
                put(2, 15, conv3(FF, xnf, 9, 1, "FF"))
                put(2, 17, conv3(GG, xnb, 18, 2, "GG"))
                put(2, 19, conv3(HH, bk[:, :, :, PAD:PAD + W], 27, 3, "HH"))
                put(2, 21, cos1_step)
                put(3, 7, cos2_step)
                put(3, 9, cos3_step)
                put(3, 11, minmax1_step)
                put(3, 13, minmax2_step)
                put(3, 15, minmax3_step)
                put(3, 17, fuse1_step)
                put(3, 19, fuse2_step)
                put(3, 21, conv3(xnb, xnf, 36, None, "ffo"))  # ffo into xnb
                put(4, 1, ffdma_step)

                # ---------- the loop ----------
                groups = []
                nt0 = 0
                while nt0 < NT:
                    g = min(GRP, NT - nt0)
                    groups.append(list(range(nt0, nt0 + g)))
                    nt0 += g

                for c, (off, n) in enumerate(PCS):
                    sl = slice(off, off + n)
                    num_ps = ps_num.tile([4, 512], f32, tag="num",
                                         name=f"num_{c}")
                    for gi, nts in enumerate(groups):
                        w = len(nts) * 512
                        sc = ps_sc.tile([128, w], f32, tag="sc",
                                        name=f"sc_{c}_{gi}")
                        for u, nt in enumerate(nts):
                            r, m = nt % NRG, nt // NRG
                            nc.tensor.matmul(
                                sc[:, u * 512:u * 512 + n],
                                k4_sb[32 * r:32 * r + 3, 128 * m:128 * m + 128],
                                q4_sb[32 * r:32 * r + 3, sl],
                                tile_position=(32 * r, 0),
                                start=True, stop=True)
                        ex = attn.tile([128, GRP * 512], f32r, tag="ex",
                                       bufs=3, name=f"ex_{c}_{gi}")
                        if len(nts) > 1:
                            sc_v = sc.rearrange("p (g c) -> p g c",
                                                g=len(nts))[:, :, 0:n]
                            nc.scalar.activation(ex[:, 0:len(nts) * n], sc_v,
                                                 AF.Exp)
                        else:
                            nc.scalar.activation(ex[:, 0:n], sc[:, 0:n],
                                                 AF.Exp)
                        for u, nt in enumerate(nts):
                            nc.tensor.matmul(
                                num_ps[:, 0:n], v_sb4[:, nt, :],
                                ex[:, u * n:(u + 1) * n],
                                start=(nt == 0), stop=(nt == NT - 1))
                        for fn in slot.get((c, gi), ()):
                            fn()
                    # ---- chunk epilogue: att = num[0:3] / num[3] ----
                    # DVE copies num out of PSUM (frees the bank for the next
                    # chunk's accumulation after ~0.7us). The den broadcast
                    # (row-3 selector matmul), reciprocal and final mul are
                    # deferred into the next chunk so they never stall the
                    # PE score stream. (A DVE op cannot read a lone
                    # partition 3, hence the selector matmul.)
                    num_sb = attn.tile([4, 512], f32r, tag="numsb", bufs=2,
                                       name=f"numsb_{c}")
                    nc.vector.tensor_copy(num_sb[:, 0:n], num_ps[:, 0:n])

                    def ep2_step(c=c, n=n, num_sb=num_sb):
                        d3p = aux_tile(3, 512, f"d3p_{c}")
                        nc.tensor.matmul(d3p[:, 0:n], sel43_sb,
                                         num_sb[:, 0:n],
                                         start=True, stop=True)
                        r3i = attn.tile([3, 512], f32, tag="r3i", bufs=2,
                                        name=f"r3i_{c}")
                        nc.vector.reciprocal(r3i[:, 0:n], d3p[:, 0:n])
                        att_t = tl.tile([3, 512], f32r, tag="att",
                                        name=f"att_{c}")
                        nc.vector.tensor_mul(att_t[:, 0:n], num_sb[0:3, 0:n],
                                             r3i[:, 0:n])
                        tail_d[c] = {"att": att_t}

                    if c < len(PCS) - 1:
                        put(c + 1, 0, ep2_step)
                    else:
                        ep2_step()

                # tail for the last chunk
                for fn in tail_steps(4):
                    fn()

    nc.compile()
    return nc


_CACHED = {}


def _prepare_in_maps(inputs):
    f = lambda k: np.asarray(inputs[k], np.float32)
    front, back = f("front"), f("back")
    bg = f("bg_embed")                      # [3, 8192]
    q_w, k_w, v_w = f("q_w"), f("k_w"), f("v_w")
    mi_w1, mi_b1 = f("mi_w1"), f("mi_b1")
    mi_w2, mi_b2 = f("mi_w2"), f("mi_b2")
    mo_w1, mo_b1 = f("mo_w1"), f("mo_b1")
    mo_w2, mo_b2 = f("mo_w2"), f("mo_b2")
    n1_g, n1_b, n2_g, n2_b = f("n1_g"), f("n1_b"), f("n2_g"), f("n2_b")
    e_w, e_b = f("e_w"), f("e_b")
    f_w, f_b = f("f_w"), f("f_b")
    g_w, g_b = f("g_w"), f("g_b")
    h_w, h_b = f("h_w"), f("h_b")
    fuse_w, fuse_b = f("fuse_w"), f("fuse_b")

    # ---- host-side weight repacking (tiny, O(n_embed * d)) ----
    kT = (k_w @ bg) * SCALE                                   # [3, NE]
    # row-group-packed k: tile nt -> row group r = nt % 3, col block nt // 3
    NRG, NKB = 3, (NT + 2) // 3
    k4 = np.zeros((3 * NRG, NKB * 128), np.float32)
    for nt in range(NT):
        r, m = nt % NRG, nt // NRG
        k4[3 * r:3 * r + 3, 128 * m:128 * (m + 1)] = \
            kT[:, nt * 128:(nt + 1) * 128]
    v = bg.T @ v_w.T                                          # [NE, 3]
    v_ext = np.concatenate([v, np.ones((NE, 1), np.float32)], 1)
    v_np = np.ascontiguousarray(
        v_ext.reshape(NT, 128, 4).transpose(1, 0, 2).reshape(128, NT * 4))
    hh, ww = np.meshgrid(np.arange(H), np.arange(H), indexing="ij")
    band = (np.abs(hh - ww) <= PAD).astype(np.float32)
    w2c = mi_w2 - mi_w2.mean(0, keepdims=True)
    b2c = mi_b2 - mi_b2.mean()
    mo2c = mo_w2 - mo_w2.mean(0, keepdims=True)
    mob2c = mo_b2 - mo_b2.mean()
    cw = np.concatenate([e_w.ravel(), f_w.ravel(), g_w.ravel(),
                         h_w.ravel(), fuse_w[:, 3:6].ravel()])
    cbias = np.concatenate(
        [np.tile(np.repeat(bb, W), B) for bb in (e_b, f_b, g_b, h_b)])

    wpack = np.zeros((6, 33), np.float32)
    wpack[0:6, 0:3] = w2c.T
    wpack[0:6, 3:6] = mo2c.T
    wpack[0:3, 6:12] = mi_w1.T
    wpack[0:3, 12:15] = q_w.T
    wpack[0:3, 15:21] = mo_w1.T
    wpack[0:3, 21:24] = fuse_w[:, 0:3].T
    wpack[0:3, 24:27] = 1.0 / 3.0
    wpack[0:1, 27:30] = 1.0
    wpack[3, 30:33] = 1.0
    vpack = np.zeros((6, 10), np.float32)
    vpack[0:6, 0] = mi_b1
    vpack[0:6, 1] = mo_b1
    vpack[0:3, 2] = b2c
    vpack[0:3, 3] = n1_g
    vpack[0:3, 4] = n1_b
    vpack[0:3, 5] = mob2c
    vpack[0:3, 6] = n2_g
    vpack[0:3, 7] = n2_b
    vpack[0:3, 8] = fuse_b
    vpack[0:3, 9] = 1e-5

    common = dict(
        front=front, back=back,
        k4=k4, v_sb=v_np,
        band=band,
        wpack=wpack, vpack=vpack,
        cw=np.ascontiguousarray(cw, np.float32),
        cbias=np.ascontiguousarray(cbias, np.float32),
    )
    common = {k: np.ascontiguousarray(v2, np.float32)
              for k, v2 in common.items()}

    in_maps = []
    for i in range(N_CORES):
        sl = front[:, :, HSL * i:HSL * (i + 1), :]          # [B,3,12,96]
        xcm = np.ascontiguousarray(
            sl.transpose(1, 0, 2, 3).reshape(D, PIX), np.float32)
        in_maps.append(dict(common, front_cm=xcm))
    return in_maps


def _gather_output(res):
    out = np.array(res.results[0]["ff_full"], np.float32)
    for i in range(N_CORES):
        oa = res.results[i]["out_a"].reshape(D, B, HSL, W)
        out[:, :, HSL * i:HSL * (i + 1), :] += oa.transpose(1, 0, 2, 3)
    return out


def kernel(**inputs):
    import sys
    if "/opt/trn_rl_repo" not in sys.path:
        sys.path.insert(0, "/opt/trn_rl_repo")
    from concourse.bass_utils import run_bass_kernel_spmd

    in_maps = _prepare_in_maps(inputs)
    if "nc" not in _CACHED:
        _CACHED["nc"] = _build_program()
    nc = _CACHED["nc"]

    res = run_bass_kernel_spmd(nc, in_maps, core_ids=list(range(N_CORES)))
    return _gather_output(res)


# revision 53
# speedup vs baseline: 380.0233x; 1.0181x over previous
"""Trainium2 Bass kernel for nn_FKRM_85839216378385 (vq_codebook).

Strategy (8 NeuronCores, SPMD):
  - Attention branch ([B*HW, n_embed] softmax-attention over an 8192-entry
    codebook) is sharded over PIXELS: core i handles image rows
    [12*i, 12*i+12) of both batches = 2304 pixels, attending over the full
    codebook. The softmax is fused (never materialized in HBM): scores^T are
    built codebook-major ([128 codes x pix] tiles, 3 tiles per 3-bank PSUM
    buffer) with row-group-packed K=3 matmuls, exp'd on the scalar engine
    straight out of PSUM in 1536-wide calls, and contracted with
    v_ext = [v | 1] so the softmax numerator and denominator come out of one
    PSUM accumulation.
  - The program is software-pipelined around the Activation engine (the
    bottleneck: ~123us of exp work per core at 0.83ns/col): the PSF
    image-fusion branch (replicated; needs global per-batch min/max) and the
    per-chunk mlp_out tails are emitted as small filler steps between score
    tile groups so they run on DVE/Pool/PE while Activation streams exps.
  - Only Exp/Ln/Gelu activation functions are used, ordered so the table
    loads happen exactly twice (gelu set once at the head, natural_log_exp
    for everything after).
  - Weight-only transforms (k = k_w @ bg_embed etc.) are repacked on host.
"""

import numpy as np

N_CORES = 8
B, C, H, W = 2, 3, 96, 96
D = 3
NE = 8192
NWIN = 7
PAD = NWIN // 2          # 3
WP = W + 2 * PAD         # 102
HSL = H // N_CORES       # 12 rows per core (per batch)
PIX = B * HSL * W        # 2304 pixels per core
BCW = B * C * W          # 576
NVAR = float(NWIN * NWIN)          # 49
SCALE = float(D) ** -0.5
PCS = [(0, 256), (256, 512), (768, 512), (1280, 512), (1792, 512)]
NT = NE // 128           # 64 codebook tiles of 128
NRG = 3                  # k row groups (PE col-tile dst must be 0/32/64)
NKB = (NT + NRG - 1) // NRG   # 22 column blocks in the packed k
GRP = 3                  # score tiles per PSUM buffer (3 banks)


def _build_program(reps=1):
    import sys
    if "/opt/trn_rl_repo" not in sys.path:
        sys.path.insert(0, "/opt/trn_rl_repo")
    import concourse.bass as bass
    import concourse.mybir as mybir
    import concourse.tile as tile
    from concourse import bacc
    import concourse.bass_isa as bass_isa
    from contextlib import ExitStack

    f32 = mybir.dt.float32
    f32r = mybir.dt.float32r
    u32 = mybir.dt.uint32
    AF = mybir.ActivationFunctionType
    ALU = mybir.AluOpType
    AX = mybir.AxisListType
    ROP = bass_isa.ReduceOp

    nc = bacc.Bacc("TRN2", target_bir_lowering=False, debug=False,
                   num_devices=N_CORES)

    # ---------------- dram I/O ----------------
    d_front = nc.dram_tensor("front", [B, C, H, W], f32, kind="ExternalInput")
    d_back = nc.dram_tensor("back", [B, C, H, W], f32, kind="ExternalInput")
    d_xcm = nc.dram_tensor("front_cm", [D, PIX], f32r, kind="ExternalInput")
    d_k4 = nc.dram_tensor("k4", [3 * NRG, NKB * 128], f32r,
                          kind="ExternalInput")
    d_v = nc.dram_tensor("v_sb", [128, NT * 4], f32r, kind="ExternalInput")
    d_band = nc.dram_tensor("band", [H, H], f32r, kind="ExternalInput")
    # all small weight matrices packed into one [6, 30] tensor (one DMA):
    # [0:6,0:3]=w2cT [0:6,3:6]=mo2cT [0:3,6:12]=w1T [0:3,12:15]=qwT
    # [0:3,15:21]=mo1T [0:3,21:24]=fuse3T [0:3,24:27]=ones33
    # [0:1,27:30]=ones13 [0:4,30:33]=sel43 (row-3 selector)
    d_wpack = nc.dram_tensor("wpack", [6, 33], f32r, kind="ExternalInput")
    # all bias/scale vectors packed into one [6, 10] tensor (one DMA):
    # cols: b1, mob1, b2c, n1g, n1b, mob2c, n2g, n2b, fuseb, eps3
    d_vpack = nc.dram_tensor("vpack", [6, 10], f32, kind="ExternalInput")
    d_cw = nc.dram_tensor("cw", [45], f32, kind="ExternalInput")
    d_cbias = nc.dram_tensor("cbias", [4 * BCW], f32, kind="ExternalInput")

    d_oa = nc.dram_tensor("out_a", [D, PIX], f32, kind="ExternalOutput")
    d_ff = nc.dram_tensor("ff_full", [B, C, H, W], f32, kind="ExternalOutput")

    def hbcw_ap(handle, b):
        """AP over one batch of a [B,C,H,W] dram tensor ordered (h | c, w)."""
        a = handle[:, :, :, :]
        return bass.AP(tensor=a.tensor, offset=a.offset + b * C * H * W,
                       ap=[[W, H], [H * W, C], [1, W]])

    def col_ap(handle, n):
        """[n] dram vector viewed as [n, 1] (one element per partition)."""
        a = handle[:]
        return bass.AP(tensor=a.tensor, offset=a.offset, ap=[[1, n], [0, 1]])

    def bcast_ap(handle, n):
        """[n] dram vector broadcast across 96 partitions -> [96, n]."""
        a = handle[:]
        return bass.AP(tensor=a.tensor, offset=a.offset, ap=[[0, 96], [1, n]])

    with tile.TileContext(nc) as tc, ExitStack() as ctx:
        consts = ctx.enter_context(tc.tile_pool(name="consts", bufs=1))
        psf = ctx.enter_context(tc.tile_pool(name="psf", bufs=1))
        psft = ctx.enter_context(tc.tile_pool(name="psft", bufs=2))
        mlp = ctx.enter_context(tc.tile_pool(name="mlp", bufs=1))
        attn = ctx.enter_context(tc.tile_pool(name="attn", bufs=3))
        tl = ctx.enter_context(tc.tile_pool(name="tl", bufs=2))

        # ---------------- constants to SBUF ----------------
        # head-critical loads (X, wpack, vpack) lead the SP queue; the big
        # loop constants (k4, v) go on the gpsimd queue so nothing blocks
        # the mlp_in head pipeline.
        X_sb = mlp.tile([D, PIX], f32r, tag="X")
        nc.sync.dma_start(out=X_sb, in_=d_xcm[:, :])
        wp_sb = consts.tile([6, 33], f32r)
        nc.sync.dma_start(out=wp_sb, in_=d_wpack[:, :])
        vp_sb = consts.tile([6, 10], f32)
        nc.sync.dma_start(out=vp_sb, in_=d_vpack[:, :])
        w2cT_sb = wp_sb[0:6, 0:3]
        mo2cT_sb = wp_sb[0:6, 3:6]
        w1T_sb = wp_sb[0:3, 6:12]
        qwT_sb = wp_sb[0:3, 12:15]
        mo1T_sb = wp_sb[0:3, 15:21]
        fuse3_sb = wp_sb[0:3, 21:24]
        ones33_sb = wp_sb[0:3, 24:27]
        ones13_sb = wp_sb[0:1, 27:30]
        sel43_sb = wp_sb[0:4, 30:33]
        b1_sb = vp_sb[0:6, 0:1]
        mob1_sb = vp_sb[0:6, 1:2]
        b2c_sb = vp_sb[0:3, 2:3]
        n1g_sb = vp_sb[0:3, 3:4]
        n1b_sb = vp_sb[0:3, 4:5]
        mob2c_sb = vp_sb[0:3, 5:6]
        n2g_sb = vp_sb[0:3, 6:7]
        n2b_sb = vp_sb[0:3, 7:8]
        fuseb_sb = vp_sb[0:3, 8:9]
        eps3_sb = vp_sb[0:3, 9:10]
        k4_sb = consts.tile([67, NKB * 128], f32r)
        for g in range(NRG):
            nc.gpsimd.dma_start(out=k4_sb[32 * g:32 * g + 3, :],
                                in_=d_k4[3 * g:3 * g + 3, :])
        v_sb = consts.tile([128, NT * 4], f32r)
        nc.gpsimd.dma_start(out=v_sb[:, 0:NT * 2], in_=d_v[:, 0:NT * 2])
        nc.gpsimd.dma_start(out=v_sb[:, NT * 2:], in_=d_v[:, NT * 2:])
        v_sb4 = v_sb.rearrange("p (n f) -> p n f", f=4)
        band_sb = consts.tile([H, H], f32r)
        nc.gpsimd.dma_start(out=band_sb, in_=d_band[:, :])
        cw_sb = consts.tile([96, 45], f32)
        nc.gpsimd.dma_start(out=cw_sb, in_=bcast_ap(d_cw, 45))
        cbias_sb = consts.tile([96, 4 * BCW], f32)
        nc.gpsimd.dma_start(out=cbias_sb, in_=bcast_ap(d_cbias, 4 * BCW))
        # fast-inverse-sqrt magic constant, replicated for tensor_sub
        magic_sb = consts.tile([96, BCW], u32)
        nc.vector.memset(magic_sb, 0x5F3759DF)

        def emit_rsqrt(eng, dst, src, mk_tmp, newton=1):
            """dst = 1/sqrt(src) via the u32 bit trick + Newton steps.

            All ops run on `eng` (DVE or gpsimd). src/dst are f32 APs with
            identical [p, n] shape, n <= BCW, p <= 96, contiguous innermost.
            mk_tmp() yields scratch f32 APs of the same shape.
            """
            p, n = src.partition_size(), src.free_size()
            y = dst
            yu = y.bitcast(u32)
            # scalar-operand ops only lower on DVE (Pool fails the codegen
            # engine check for TensorScalar); plain muls may run on `eng`
            nc.vector.tensor_scalar(yu, src.bitcast(u32), 1, None,
                                    ALU.logical_shift_right)
            eng.tensor_sub(yu, magic_sb[0:p, 0:n], yu)
            for _ in range(newton):
                t = mk_tmp()
                eng.tensor_mul(t, y, y)
                eng.tensor_mul(t, t, src)
                nc.vector.tensor_scalar(t, t, -0.5, 1.5, ALU.mult, ALU.add)
                eng.tensor_mul(y, y, t)

        for _rep in range(reps):
            # =========================================================
            # PSF input staging (independent of everything else)
            # layout L0: [h=96 partitions | (b, c, w)]
            # =========================================================
            fr = psf.tile([96, B, C, WP], f32, tag="fr")
            bk = psf.tile([96, B, C, WP], f32, tag="bk")
            for t in (fr, bk):
                nc.vector.memset(t[:, :, :, 0:PAD], 0.0)
                nc.vector.memset(t[:, :, :, PAD + W:], 0.0)
            fb_eng = [nc.gpsimd, nc.gpsimd, nc.sync, nc.sync]
            for b in range(B):
                fb_eng[b].dma_start(out=fr[:, b, :, PAD:PAD + W],
                                    in_=hbcw_ap(d_front, b))
                fb_eng[2 + b].dma_start(out=bk[:, b, :, PAD:PAD + W],
                                        in_=hbcw_ap(d_back, b))

            # =========================================================
            # head: mlp_in + ln1 + q, chunked (channel-major [d | pix]).
            # All 5 gelus run back-to-back on Act (one gelu table load);
            # the rest of the head uses no Act at all (DVE bit-trick rsqrt
            # for the ln1 rstd). Only chunk 0 completes before the
            # attention loop; chunks 1-4 are emitted as in-loop fillers.
            # =========================================================
            q4_sb = mlp.tile([67, PIX], f32r, tag="q4")
            hsb = {}

            def make_head_steps(c, alloc):
                """Three filler steps computing q4 for chunk c."""
                off, n = PCS[c]
                st = {}

                def mk_tmp():
                    return tl.tile([6, 512], f32, tag="t_tmp", bufs=3,
                                   name=f"nt_{c}")[0:3, 0:n]

                def hd1():
                    yp = alloc(3, 512, f"yps_{c}")
                    nc.tensor.matmul(yp[:, 0:n], w2cT_sb, hsb[c][:, 0:n],
                                     start=True, stop=True)
                    y_t = tl.tile([3, 512], f32, tag="y", name=f"y_{c}")
                    nc.vector.tensor_scalar(y_t[:, 0:n], yp[:, 0:n],
                                            b2c_sb, None, ALU.add)
                    sq_t = tl.tile([6, 512], f32r, tag="t_tmp", bufs=3,
                                   name=f"sq_{c}")[0:3, :]
                    nc.vector.tensor_mul(sq_t[:, 0:n], y_t[:, 0:n],
                                         y_t[:, 0:n])
                    st["y"], st["sq"] = y_t, sq_t

                def hd2():
                    vp = alloc(3, 512, f"vps_{c}")
                    nc.tensor.matmul(vp[:, 0:n], ones33_sb,
                                     st["sq"][:, 0:n], start=True, stop=True)
                    u_t = tl.tile([6, 512], f32, tag="t_tmp", bufs=3,
                                  name=f"u_{c}")[0:3, :]
                    nc.vector.tensor_scalar_add(u_t[:, 0:n], vp[:, 0:n], 1e-5)
                    r_t = tl.tile([3, 512], f32, tag="y", name=f"rst_{c}")
                    emit_rsqrt(nc.vector, r_t[:, 0:n], u_t[:, 0:n], mk_tmp,
                               newton=1)
                    st["r"] = r_t

                def hd3():
                    x2_t = tl.tile([6, 512], f32r, tag="t_tmp", bufs=3,
                                   name=f"x2_{c}")[0:3, :]
                    nc.vector.tensor_mul(x2_t[:, 0:n], st["y"][:, 0:n],
                                         st["r"][:, 0:n])
                    nc.vector.tensor_scalar(x2_t[:, 0:n], x2_t[:, 0:n],
                                            n1g_sb, n1b_sb, ALU.mult, ALU.add)
                    qp = alloc(3, 512, f"qps_{c}")
                    nc.tensor.matmul(qp[:, 0:n], qwT_sb, x2_t[:, 0:n],
                                     start=True, stop=True)
                    # replicate q to the three k row groups (PE column
                    # tiling is rejected by codegen; DVE writes to 32-aligned
                    # partition bases are fine)
                    for g in range(NRG):
                        nc.vector.tensor_copy(
                            q4_sb[32 * g:32 * g + 3, off:off + n], qp[:, 0:n])

                return [hd1, hd2, hd3]

            with tc.tile_pool(name="ps_head", bufs=1, space="PSUM") as psH:
                hps = {}

                def emit_gelu(c):
                    off, n = PCS[c]
                    # bufs=5: h chunks 1-4 stay live until their in-loop
                    # y-matmul fillers
                    ht = tl.tile([6, 512], f32r, tag="hsb", bufs=5,
                                 name=f"h_{c}")
                    nc.scalar.activation(ht[:, 0:n], hps[c][:, 0:n],
                                         AF.Gelu_apprx_tanh, bias=b1_sb)
                    hsb[c] = ht

                for c, (off, n) in enumerate(PCS):
                    hp = psH.tile([6, 512], f32, tag="hps", bufs=2,
                                  name=f"hps_{c}")
                    nc.tensor.matmul(hp[:, 0:n], w1T_sb, X_sb[:, off:off + n],
                                     start=True, stop=True)
                    hps[c] = hp
                    if c >= 1:
                        emit_gelu(c - 1)
                emit_gelu(len(PCS) - 1)

                # preload the exp table while the head pipeline runs, so
                # the first attention exp doesn't pay the 1.28us table load
                dume = tl.tile([1, 8], f32, tag="dume", bufs=1)
                nc.scalar.activation(dume[:, 0:1], vp_sb[0:1, 0:1], AF.Exp)

                def psh_alloc(p, w, name):
                    return psH.tile([p, w], f32, tag="hd", bufs=2, name=name)

                for fn in make_head_steps(0, psh_alloc):
                    fn()

            # =========================================================
            # PSF stage A: squares (Pool) + 7-tap horizontal box (DVE)
            # =========================================================
            sqf = psf.tile([96, B, C, WP], f32, tag="sqf")
            sqb = psf.tile([96, B, C, WP], f32, tag="sqb")
            nc.gpsimd.tensor_mul(sqf, fr, fr)
            nc.gpsimd.tensor_mul(sqb, bk, bk)

            def hbox_half(dst, src, first):
                """7-tap horizontal box sum, split in two filler halves."""
                def fn():
                    if first:
                        nc.vector.tensor_add(dst, src[:, :, :, 0:W],
                                             src[:, :, :, 1:1 + W])
                        for j in range(2, 4):
                            nc.vector.tensor_add(dst, dst,
                                                 src[:, :, :, j:j + W])
                    else:
                        for j in range(4, NWIN):
                            nc.vector.tensor_add(dst, dst,
                                                 src[:, :, :, j:j + W])
                return fn

            brh_f = psf.tile([96, B, C, W], f32r, tag="brh_f")
            brh_f2 = psf.tile([96, B, C, W], f32r, tag="brh_f2")
            brh_b = psf.tile([96, B, C, W], f32r, tag="brh_b")
            brh_b2 = psf.tile([96, B, C, W], f32r, tag="brh_b2")

            # PSF result tiles (filled by filler steps inside the loop;
            # vbox overwrites brh_* in place, so s1f==brh_f etc.)
            m_f = psf.tile([96, B, C, W], f32, tag="m_f")
            r_f = psf.tile([96, B, C, W], f32, tag="r_f")
            m_b = psf.tile([96, B, C, W], f32, tag="m_b")
            r_b = psf.tile([96, B, C, W], f32, tag="r_b")
            sd_b = psf.tile([96, B, C, W], f32, tag="sd_b")
            xnf = psf.tile([96, B, C, W], f32, tag="xnf")
            xnb = psf.tile([96, B, C, W], f32, tag="xnb")
            xad = psf.tile([96, B, C, W], f32, tag="xad")
            EE = psf.tile([96, B, C, W], f32, tag="EE")
            FF = psf.tile([96, B, C, W], f32, tag="FF")
            GG = psf.tile([96, B, C, W], f32, tag="GG")
            HH = psf.tile([96, B, C, W], f32, tag="HH")
            dot = psf.tile([96, B, W], f32, tag="dot")
            f2 = psf.tile([96, B, W], f32, tag="f2")
            g2 = psf.tile([96, B, W], f32, tag="g2")
            S = psf.tile([96, B, W], f32, tag="S")
            Sn = psf.tile([96, B, W], f32, tag="Sn")

            # =========================================================
            # attention main loop, with PSF + mlp_out tails as fillers
            # =========================================================
            with tc.tile_pool(name="ps_sc", bufs=2, space="PSUM") as ps_sc, \
                 tc.tile_pool(name="ps_num", bufs=1, space="PSUM") as ps_num, \
                 tc.tile_pool(name="ps_aux", bufs=1, space="PSUM") as ps_aux:

                # ---------- filler step definitions ----------
                def aux_tile(p, w, name):
                    return ps_aux.tile([p, w], f32, tag="aux", name=name)

                def vbox_step(srcdst, idx):
                    """srcdst = Band^T @ srcdst over the h (partition) axis,
                    in place (each 288-col half is matmul'd then written
                    back)."""
                    def fn():
                        sflat = srcdst.rearrange("p b c w -> p (b c w)")
                        for half in range(2):
                            slh = slice(half * 288, half * 288 + 288)
                            bp = aux_tile(96, 288, f"vb_{idx}_{half}")
                            nc.tensor.matmul(bp, band_sb, sflat[:, slh],
                                             start=True, stop=True)
                            nc.vector.tensor_copy(sflat[:, slh], bp)
                    return fn

                def stats1_step(s1, s2, tagp):
                    def fn():
                        v_t = psft.tile([96, B, C, W], f32, tag="st_v",
                                        bufs=1, name=f"v_{tagp}")
                        u_t = psft.tile([96, B, C, W], f32, tag="st_u",
                                        bufs=1, name=f"u_{tagp}")
                        nc.vector.tensor_mul(v_t, s1, s1)
                        nc.vector.tensor_scalar_mul(
                            v_t, v_t, -1.0 / (NVAR * (NVAR - 1)))
                        nc.vector.tensor_scalar_mul(u_t, s2, 1.0 / (NVAR - 1))
                        nc.vector.tensor_add(v_t, v_t, u_t)
                        self_d[tagp] = v_t
                    return fn

                self_d = {}

                def stats2_step(s1, mean_t, rstd_t, sd_t, tagp):
                    def fn():
                        # rstd = rsqrt(var) on DVE (no act-table traffic);
                        # sd (needed only for 'back') = var * rstd
                        v_t = self_d[tagp]
                        vfl = v_t.rearrange("p b c w -> p (b c w)")
                        rfl = rstd_t.rearrange("p b c w -> p (b c w)")

                        def mk_tmp():
                            t = psft.tile([96, B, C, W], f32, tag="st_r",
                                          bufs=1, name=f"rs_{tagp}")
                            return t.rearrange("p b c w -> p (b c w)")

                        emit_rsqrt(nc.vector, rfl, vfl, mk_tmp, newton=1)
                        if sd_t is not None:
                            nc.vector.tensor_mul(sd_t, v_t, rstd_t)
                        nc.vector.tensor_scalar_mul(mean_t, s1, 1.0 / NVAR)
                    return fn

                def mvn_step(dst, src, m_t, r_t):
                    def fn():
                        nc.vector.tensor_sub(dst, src[:, :, :, PAD:PAD + W], m_t)
                        nc.vector.tensor_mul(dst, dst, r_t)
                    return fn

                def adain_step():
                    nc.vector.tensor_mul(xad, xnf, sd_b)
                    nc.vector.tensor_add(xad, xad, m_b)

                def conv3(dst, src, wbase, bias_idx, name):
                    def fn():
                        for co in range(3):
                            dco = dst[:, :, co, :]
                            t = psft.tile([96, B, W], f32, tag="conv_t",
                                          name=f"cv_{name}_{co}")
                            nc.vector.tensor_scalar_mul(
                                dco, src[:, :, 0, :],
                                cw_sb[:, wbase + co * 3:wbase + co * 3 + 1])
                            nc.vector.tensor_scalar_mul(
                                t, src[:, :, 1, :],
                                cw_sb[:, wbase + co * 3 + 1:wbase + co * 3 + 2])
                            nc.vector.tensor_add(dco, dco, t)
                            nc.vector.tensor_scalar_mul(
                                t, src[:, :, 2, :],
                                cw_sb[:, wbase + co * 3 + 2:wbase + co * 3 + 3])
                            nc.vector.tensor_add(dco, dco, t)
                        if bias_idx is not None:
                            dflat = dst.rearrange("p b c w -> p (b c w)")
                            nc.vector.tensor_add(
                                dflat, dflat,
                                cbias_sb[:, bias_idx * BCW:(bias_idx + 1) * BCW])
                    return fn

                def cos1_step():
                    tmc = psft.tile([96, B, W], f32, tag="cos_t", name="tmc1")
                    nc.vector.tensor_mul(dot, FF[:, :, 0, :], GG[:, :, 0, :])
                    nc.vector.tensor_mul(f2, FF[:, :, 0, :], FF[:, :, 0, :])
                    nc.vector.tensor_mul(g2, GG[:, :, 0, :], GG[:, :, 0, :])
                    nc.vector.tensor_mul(tmc, FF[:, :, 1, :], GG[:, :, 1, :])
                    nc.vector.tensor_add(dot, dot, tmc)

                def cos2_step():
                    tmc = psft.tile([96, B, W], f32, tag="cos_t", name="tmc2")
                    nc.vector.tensor_mul(tmc, FF[:, :, 1, :], FF[:, :, 1, :])
                    nc.vector.tensor_add(f2, f2, tmc)
                    nc.vector.tensor_mul(tmc, GG[:, :, 1, :], GG[:, :, 1, :])
                    nc.vector.tensor_add(g2, g2, tmc)
                    nc.vector.tensor_mul(tmc, FF[:, :, 2, :], GG[:, :, 2, :])
                    nc.vector.tensor_add(dot, dot, tmc)

                def cos3_step():
                    tmc = psft.tile([96, B, W], f32, tag="cos_t", name="tmc3")
                    nc.vector.tensor_mul(tmc, FF[:, :, 2, :], FF[:, :, 2, :])
                    nc.vector.tensor_add(f2, f2, tmc)
                    nc.vector.tensor_mul(tmc, GG[:, :, 2, :], GG[:, :, 2, :])
                    nc.vector.tensor_add(g2, g2, tmc)
                    nc.vector.tensor_mul(f2, f2, g2)          # F2*G2
                    # 1/(Fn*Gn) = rsqrt(F2*G2) on DVE
                    f2fl = f2.rearrange("p b w -> p (b w)")
                    g2fl = g2.rearrange("p b w -> p (b w)")

                    def mk_tmp():
                        t = psft.tile([96, B, W], f32, tag="cos_t",
                                      name="tmc4")
                        return t.rearrange("p b w -> p (b w)")

                    emit_rsqrt(nc.vector, g2fl, f2fl, mk_tmp, newton=1)
                    nc.vector.tensor_mul(S, dot, g2)

                mm_d = {}

                def minmax1_step():
                    rmx = psft.tile([96, 2], f32, tag="rmx", name="rmx")
                    rmn = psft.tile([96, 2], f32, tag="rmn", name="rmn")
                    nS = psft.tile([96, B, W], f32, tag="nS", name="nS")
                    nc.vector.tensor_scalar_mul(nS, S, -1.0)
                    nc.vector.tensor_reduce(rmx, S, axis=AX.X, op=ALU.max)
                    nc.vector.tensor_reduce(rmn, nS, axis=AX.X, op=ALU.max)
                    mm_d["rmx"], mm_d["rmn"] = rmx, rmn

                def minmax2_step():
                    nmn_bc = psft.tile([96, 2], f32, tag="nmn_bc", name="nmn_bc")
                    mx_bc = psft.tile([96, 2], f32, tag="mx_bc", name="mx_bc")
                    dd_bc = psft.tile([96, 2], f32, tag="dd_bc", name="dd_bc")
                    ri_bc = psft.tile([96, 2], f32, tag="ri_bc", name="ri_bc")
                    nc.gpsimd.partition_all_reduce(mx_bc, mm_d["rmx"], 96, ROP.max)
                    nc.gpsimd.partition_all_reduce(nmn_bc, mm_d["rmn"], 96, ROP.max)
                    nc.vector.tensor_add(dd_bc, mx_bc, nmn_bc)    # max - min
                    nc.vector.reciprocal(ri_bc, dd_bc)
                    mm_d["nmn"], mm_d["ri"] = nmn_bc, ri_bc

                def minmax3_step():
                    for b in range(B):
                        nc.vector.tensor_scalar(
                            Sn[:, b, :], S[:, b, :],
                            mm_d["nmn"][:, b:b + 1], mm_d["ri"][:, b:b + 1],
                            ALU.add, ALU.mult)

                def fuse1_step():
                    nc.vector.tensor_sub(xnf, EE, HH)   # reuse xnf as scratch
                    for cc in range(3):
                        nc.vector.tensor_mul(xnf[:, :, cc, :],
                                             xnf[:, :, cc, :], Sn)

                def fuse2_step():
                    ff_flat = xnf.rearrange("p b c w -> p (b c w)")
                    hh_flat = HH.rearrange("p b c w -> p (b c w)")
                    nc.vector.tensor_add(ff_flat, ff_flat, hh_flat)

                def ffdma_step():
                    for b in range(B):
                        nc.sync.dma_start(out=hbcw_ap(d_ff, b),
                                          in_=xnb[:, b, :, :])

                # ---------- mlp_out tail steps for one chunk ----------
                # transient [<=6, 512] scratch shares one 3-deep rotating tag
                tail_d = {}

                def tmp_tile(c, nm):
                    return tl.tile([6, 512], f32, tag="t_tmp", bufs=3,
                                   name=f"{nm}_{c}")

                def t0_step(c):
                    def fn():
                        off, n = PCS[c]
                        h2p = aux_tile(6, 512, f"h2p_{c}")
                        nc.tensor.matmul(h2p[:, 0:n], mo1T_sb,
                                         tail_d[c]["att"][:, 0:n],
                                         start=True, stop=True)
                        xg = tl.tile([6, 512], f32, tag="t_xg", name=f"xg_{c}")
                        nc.vector.tensor_scalar(xg[:, 0:n], h2p[:, 0:n],
                                                mob1_sb, None, ALU.add)
                        tail_d[c]["xg"] = xg
                    return fn

                def t1_step(c):
                    def fn():
                        off, n = PCS[c]
                        d = tail_d[c]
                        eng = nc.gpsimd if c < len(PCS) - 1 else nc.vector
                        s = tmp_tile(c, "s")
                        eng.tensor_mul(s[:, 0:n], d["xg"][:, 0:n],
                                       d["xg"][:, 0:n])
                        eng.tensor_mul(s[:, 0:n], s[:, 0:n],
                                       d["xg"][:, 0:n])
                        d["cb"] = s
                    return fn

                def t2_step(c):
                    def fn():
                        off, n = PCS[c]
                        d = tail_d[c]
                        t = tmp_tile(c, "t")
                        nc.vector.tensor_scalar_mul(t[:, 0:n], d["cb"][:, 0:n],
                                                    0.044715)
                        nc.vector.tensor_add(t[:, 0:n], t[:, 0:n],
                                             d["xg"][:, 0:n])
                        e = tmp_tile(c, "e")
                        nc.scalar.activation(e[:, 0:n], t[:, 0:n], AF.Exp,
                                             scale=1.5957691216057308)
                        d["e"] = e
                    return fn

                def t3_step(c):
                    def fn():
                        off, n = PCS[c]
                        d = tail_d[c]
                        dd = tmp_tile(c, "dd")
                        nc.vector.tensor_scalar_add(dd[:, 0:n], d["e"][:, 0:n],
                                                    1.0)
                        r = tmp_tile(c, "r")
                        nc.vector.reciprocal(r[:, 0:n], dd[:, 0:n])
                        nc.vector.tensor_mul(r[:, 0:n], d["e"][:, 0:n],
                                             r[:, 0:n])
                        h2 = tl.tile([6, 512], f32r, tag="t_h2", name=f"h2_{c}")
                        nc.vector.tensor_mul(h2[:, 0:n], d["xg"][:, 0:n],
                                             r[:, 0:n])
                        d["h2"] = h2
                    return fn

                def t4_step(c):
                    def fn():
                        off, n = PCS[c]
                        d = tail_d[c]
                        y2p = aux_tile(3, 512, f"y2p_{c}")
                        nc.tensor.matmul(y2p[:, 0:n], mo2cT_sb,
                                         d["h2"][:, 0:n], start=True, stop=True)
                        y2 = tl.tile([3, 512], f32, tag="t_y2", name=f"ty2_{c}")
                        nc.vector.tensor_scalar(y2[:, 0:n], y2p[:, 0:n],
                                                mob2c_sb, None, ALU.add)
                        sq2 = tl.tile([3, 512], f32r, tag="t_sq2",
                                      name=f"tsq_{c}")
                        nc.vector.tensor_mul(sq2[:, 0:n], y2[:, 0:n],
                                             y2[:, 0:n])
                        d["y2"], d["sq2"] = y2, sq2
                    return fn

                def t5_step(c):
                    # rstd for ln2 = rsqrt(var + eps), off the Act engine
                    # (gpsimd for pipelined chunks, DVE for the last one
                    # where end-to-end latency matters)
                    def fn():
                        off, n = PCS[c]
                        d = tail_d[c]
                        eng = nc.gpsimd if c < len(PCS) - 1 else nc.vector
                        v2p = aux_tile(3, 512, f"v2p_{c}")
                        nc.tensor.matmul(v2p[:, 0:n], ones33_sb,
                                         d["sq2"][:, 0:n], start=True, stop=True)
                        u2 = tmp_tile(c, "u2")
                        # gpsimd cannot read PSUM; the eps-add stays on DVE
                        nc.vector.tensor_scalar_add(u2[0:3, 0:n], v2p[:, 0:n],
                                                    1e-5)
                        r32 = tl.tile([3, 512], f32, tag="t_r32",
                                      name=f"tr32_{c}")

                        def mk_tmp():
                            return tmp_tile(c, "rt")[0:3, 0:n]

                        emit_rsqrt(eng, r32[:, 0:n], u2[0:3, 0:n], mk_tmp,
                                   newton=1)
                        d["r32"] = r32
                    return fn

                def t6_step(c):
                    # ln2's gamma/beta are folded into fuse3/fuseb host-side
                    def fn():
                        off, n = PCS[c]
                        d = tail_d[c]
                        x5 = tl.tile([3, 512], f32r, tag="t_x5", bufs=3,
                                     name=f"x5_{c}")
                        nc.vector.tensor_mul(x5[:, 0:n], d["y2"][:, 0:n],
                                             d["r32"][:, 0:n])
                        d["x5"] = x5
                    return fn

                def t7_step(c):
                    def fn():
                        off, n = PCS[c]
                        d = tail_d[c]
                        op = aux_tile(3, 512, f"op_{c}")
                        nc.tensor.matmul(op[:, 0:n], fuse3_sb, d["x5"][:, 0:n],
                                         start=True, stop=True)
                        oa = tl.tile([3, 512], f32, tag="t_x5", bufs=3,
                                    name=f"oa_{c}")
                        nc.vector.tensor_scalar(oa[:, 0:n], op[:, 0:n],
                                                fuseb_sb, None, ALU.add)
                        nc.sync.dma_start(out=d_oa[:, off:off + n],
                                          in_=oa[:, 0:n])
                    return fn

                def t0g_step(c):
                    # last chunk only: gelu via the act table (Act is idle
                    # once the loop exps end, and the short chain trims the
                    # end-of-program latency)
                    def fn():
                        off, n = PCS[c]
                        h2p = aux_tile(6, 512, f"h2p_{c}")
                        nc.tensor.matmul(h2p[:, 0:n], mo1T_sb,
                                         tail_d[c]["att"][:, 0:n],
                                         start=True, stop=True)
                        h2 = tl.tile([6, 512], f32r, tag="t_h2",
                                     name=f"h2_{c}")
                        nc.scalar.activation(h2[:, 0:n], h2p[:, 0:n],
                                             AF.Gelu_apprx_tanh, bias=mob1_sb)
                        tail_d[c]["h2"] = h2
                    return fn

                def tail_steps(c):
                    if c == len(PCS) - 1:
                        return [t0g_step(c), t4_step(c), t5_step(c),
                                t6_step(c), t7_step(c)]
                    return [t0_step(c), t1_step(c), t2_step(c), t3_step(c),
                            t4_step(c), t5_step(c), t6_step(c), t7_step(c)]

                # nt=0 pre-tile: scored into the aux bank and exp'd ahead
                # of its chunk
                ex0_d = {}

                def pre_step(c):
                    def fn():
                        off, n = PCS[c]
                        scp = aux_tile(128, 512, f"scp_{c}")
                        nc.tensor.matmul(scp[:, 0:n], k4_sb[0:3, 0:128],
                                         q4_sb[0:3, off:off + n],
                                         tile_position=(0, 0),
                                         start=True, stop=True)
                        ex0 = tl.tile([128, 512], f32r, tag="ex0", bufs=2,
                                      name=f"ex0_{c}")
                        nc.scalar.activation(ex0[:, 0:n], scp[:, 0:n], AF.Exp)
                        ex0_d[c] = ex0
                    return fn

                # ---------- filler schedule: slot (chunk, group) -> fns ----
                slot = {}

                def put(c, gi, fn):
                    slot.setdefault((c, gi), []).append(fn)

                # head chunks 1-4: hd1/hd2 early in chunk c-1; hd3 at
                # gi 9 so its q-matmuls sit well after the rsqrt chain
                for hc in range(1, 5):
                    h1f, h2f, h3f = make_head_steps(hc, aux_tile)
                    put(hc - 1, 1, h1f)
                    put(hc - 1, 3, h2f)
                    put(hc - 1, 9, h3f)

                # mlp_out tails: t2's act-exp and t6 get extra slack so they
                # never block the in-order Act/PE streams
                TSLOTS = [4, 6, 8, 10, 12, 14, 19, 20]
                for tc_ in range(4):
                    for i, fn in enumerate(tail_steps(tc_)):
                        put(tc_ + 1, TSLOTS[i], fn)

                # next chunk's nt=0 pre-tile (aux PSUM) near this chunk's end
                for pc in range(1, 5):
                    put(pc - 1, 16, pre_step(pc))

                # PSF pipeline, odd slots
                put(0, 5, hbox_half(brh_f, fr, True))
                put(0, 7, hbox_half(brh_f, fr, False))
                put(0, 11, hbox_half(brh_f2, sqf, True))
                put(0, 13, hbox_half(brh_f2, sqf, False))
                put(0, 15, hbox_half(brh_b, bk, True))
                put(0, 17, hbox_half(brh_b, bk, False))
                put(0, 19, hbox_half(brh_b2, sqb, True))
                put(1, 5, hbox_half(brh_b2, sqb, False))
                put(1, 7, vbox_step(brh_f, 0))
                put(1, 11, vbox_step(brh_f2, 1))
                put(1, 13, vbox_step(brh_b, 2))
                put(1, 15, vbox_step(brh_b2, 3))
                put(1, 17, stats1_step(brh_f, brh_f2, "f"))
                put(1, 19, stats2_step(brh_f, m_f, r_f, None, "f"))
                put(2, 5, stats1_step(brh_b, brh_b2, "b"))
                put(2, 7, stats2_step(brh_b, m_b, r_b, sd_b, "b"))
                put(2, 11, mvn_step(xnf, fr, m_f, r_f))
                put(2, 13, mvn_step(xnb, bk, m_b, r_b))
                put(2, 15, adain_step)
                put(2, 17, conv3(EE, xad, 0, 0, "EE"))
                put(2, 19, conv3(FF, xnf, 9, 1, "FF"))
                put(3, 5, conv3(GG, xnb, 18, 2, "GG"))
                put(3, 7, conv3(HH, bk[:, :, :, PAD:PAD + W], 27, 3, "HH"))
                put(3, 11, cos1_step)
                put(3, 13, cos2_step)
                put(3, 15, cos3_step)
                put(3, 17, minmax1_step)
                put(3, 19, minmax2_step)
                put(4, 5, minmax3_step)
                put(4, 7, fuse1_step)
                put(4, 11, fuse2_step)
                put(4, 13, conv3(xnb, xnf, 36, None, "ffo"))  # ffo into xnb
                put(4, 15, ffdma_step)

                pre_step(0)()

                # ---------- the loop ----------
                # tile nt=0 of each chunk is peeled into the spare aux PSUM
                # bank and emitted during the previous chunk; the remaining
                # 63 tiles form exactly 21 groups of 3. Contractions lag one
                # group behind their scores in the PE stream, and a chunk's
                # LAST contraction (plus the whole epilogue) is flushed after
                # the NEXT chunk's second score group — the in-order PE
                # stream then never waits on the exp stream at a boundary.
                groups = [list(range(s, s + GRP)) for s in range(1, NT, GRP)]

                def contract(num_ps, ex, nts, n, last):
                    for u, nt in enumerate(nts):
                        nc.tensor.matmul(
                            num_ps[:, 0:n], v_sb4[:, nt, :],
                            ex[:, u * n:(u + 1) * n],
                            start=False, stop=(nt == NT - 1))
                    if last:
                        # epilogue part 1: free the num bank
                        c_, n_, nps = last
                        num_sb = attn.tile([4, 512], f32r, tag="numsb",
                                           bufs=2, name=f"numsb_{c_}")
                        nc.vector.tensor_copy(num_sb[:, 0:n_], nps[:, 0:n_])
                        ep_d[c_] = num_sb

                def ep2_step(c):
                    def fn():
                        off, n = PCS[c]
                        num_sb = ep_d[c]
                        d3p = aux_tile(3, 512, f"d3p_{c}")
                        nc.tensor.matmul(d3p[:, 0:n], sel43_sb,
                                         num_sb[:, 0:n],
                                         start=True, stop=True)
                        r3i = attn.tile([3, 512], f32, tag="r3i", bufs=2,
                                        name=f"r3i_{c}")
                        nc.vector.reciprocal(r3i[:, 0:n], d3p[:, 0:n])
                        att_t = tl.tile([3, 512], f32r, tag="att",
                                        name=f"att_{c}")
                        nc.vector.tensor_mul(att_t[:, 0:n], num_sb[0:3, 0:n],
                                             r3i[:, 0:n])
                        tail_d[c] = {"att": att_t}
                    return fn

                ep_d = {}
                held = None   # (num_ps, ex, nts, n, last_info) from prev grp

                for c, (off, n) in enumerate(PCS):
                    sl = slice(off, off + n)
                    num_ps = ps_num.tile([4, 512], f32, tag="num",
                                         name=f"num_{c}")
                    for gi, nts in enumerate(groups):
                        w = len(nts) * 512
                        sc = ps_sc.tile([128, w], f32, tag="sc",
                                        name=f"sc_{c}_{gi}")
                        for u, nt in enumerate(nts):
                            r, m = nt % NRG, nt // NRG
                            nc.tensor.matmul(
                                sc[:, u * 512:u * 512 + n],
                                k4_sb[32 * r:32 * r + 3, 128 * m:128 * m + 128],
                                q4_sb[32 * r:32 * r + 3, sl],
                                tile_position=(32 * r, 0),
                                start=True, stop=True)
                        ex = attn.tile([128, GRP * 512], f32r, tag="ex",
                                       bufs=3, name=f"ex_{c}_{gi}")
                        sc_v = sc.rearrange("p (g c) -> p g c",
                                            g=len(nts))[:, :, 0:n]
                        nc.scalar.activation(ex[:, 0:len(nts) * n], sc_v,
                                             AF.Exp)
                        if gi == 1:
                            if held is not None:
                                contract(*held)
                                held = None
                            # this chunk's accumulation starts with the
                            # pre-tile (start=True resets PSUM)
                            nc.tensor.matmul(num_ps[:, 0:n], v_sb4[:, 0, :],
                                             ex0_d[c][:, 0:n],
                                             start=True, stop=False)
                            contract(num_ps, exs[0], groups[0], n, None)
                        elif gi >= 2:
                            contract(num_ps, exs[gi - 1], groups[gi - 1], n,
                                     None)
                        if gi == 0:
                            exs = {}
                        exs[gi] = ex
                        for fn in slot.get((c, gi), ()):
                            fn()
                    held = (num_ps, exs[len(groups) - 1],
                            groups[len(groups) - 1], n, (c, n, num_ps))
                    if c < len(PCS) - 1:
                        put(c + 1, 2, ep2_step(c))

                # flush the last chunk's contraction + epilogue
                contract(*held)
                held = None
                ep2_step(len(PCS) - 1)()

                # tail for the last chunk
                for fn in tail_steps(4):
                    fn()

    nc.compile()
    return nc


_CACHED = {}


def _prepare_in_maps(inputs):
    f = lambda k: np.asarray(inputs[k], np.float32)
    front, back = f("front"), f("back")
    bg = f("bg_embed")                      # [3, 8192]
    q_w, k_w, v_w = f("q_w"), f("k_w"), f("v_w")
    mi_w1, mi_b1 = f("mi_w1"), f("mi_b1")
    mi_w2, mi_b2 = f("mi_w2"), f("mi_b2")
    mo_w1, mo_b1 = f("mo_w1"), f("mo_b1")
    mo_w2, mo_b2 = f("mo_w2"), f("mo_b2")
    n1_g, n1_b, n2_g, n2_b = f("n1_g"), f("n1_b"), f("n2_g"), f("n2_b")
    e_w, e_b = f("e_w"), f("e_b")
    f_w, f_b = f("f_w"), f("f_b")
    g_w, g_b = f("g_w"), f("g_b")
    h_w, h_b = f("h_w"), f("h_b")
    fuse_w, fuse_b = f("fuse_w"), f("fuse_b")

    # ---- host-side weight repacking (tiny, O(n_embed * d)) ----
    kT = (k_w @ bg) * SCALE                                   # [3, NE]
    # row-group-packed k: tile nt -> row group r = nt % 3, col block nt // 3
    NRG, NKB = 3, (NT + 2) // 3
    k4 = np.zeros((3 * NRG, NKB * 128), np.float32)
    for nt in range(NT):
        r, m = nt % NRG, nt // NRG
        k4[3 * r:3 * r + 3, 128 * m:128 * (m + 1)] = \
            kT[:, nt * 128:(nt + 1) * 128]
    v = bg.T @ v_w.T                                          # [NE, 3]
    v_ext = np.concatenate([v, np.ones((NE, 1), np.float32)], 1)
    v_np = np.ascontiguousarray(
        v_ext.reshape(NT, 128, 4).transpose(1, 0, 2).reshape(128, NT * 4))
    hh, ww = np.meshgrid(np.arange(H), np.arange(H), indexing="ij")
    band = (np.abs(hh - ww) <= PAD).astype(np.float32)
    w2c = mi_w2 - mi_w2.mean(0, keepdims=True)
    b2c = mi_b2 - mi_b2.mean()
    mo2c = mo_w2 - mo_w2.mean(0, keepdims=True)
    mob2c = mo_b2 - mo_b2.mean()
    cw = np.concatenate([e_w.ravel(), f_w.ravel(), g_w.ravel(),
                         h_w.ravel(), fuse_w[:, 3:6].ravel()])
    cbias = np.concatenate(
        [np.tile(np.repeat(bb, W), B) for bb in (e_b, f_b, g_b, h_b)])

    wpack = np.zeros((6, 33), np.float32)
    wpack[0:6, 0:3] = w2c.T
    wpack[0:6, 3:6] = mo2c.T
    wpack[0:3, 6:12] = mi_w1.T
    wpack[0:3, 12:15] = q_w.T
    wpack[0:3, 15:21] = mo_w1.T
    wpack[0:3, 21:24] = fuse_w[:, 0:3].T * n2_g[:, None]
    wpack[0:3, 24:27] = 1.0 / 3.0
    wpack[0:1, 27:30] = 1.0
    wpack[3, 30:33] = 1.0
    vpack = np.zeros((6, 10), np.float32)
    vpack[0:6, 0] = mi_b1
    vpack[0:6, 1] = mo_b1
    vpack[0:3, 2] = b2c
    vpack[0:3, 3] = n1_g
    vpack[0:3, 4] = n1_b
    vpack[0:3, 5] = mob2c
    vpack[0:3, 6] = n2_g
    vpack[0:3, 7] = n2_b
    vpack[0:3, 8] = fuse_b + fuse_w[:, 0:3] @ n2_b
    vpack[0:3, 9] = 1e-5

    common = dict(
        front=front, back=back,
        k4=k4, v_sb=v_np,
        band=band,
        wpack=wpack, vpack=vpack,
        cw=np.ascontiguousarray(cw, np.float32),
        cbias=np.ascontiguousarray(cbias, np.float32),
    )
    common = {k: np.ascontiguousarray(v2, np.float32)
              for k, v2 in common.items()}

    in_maps = []
    for i in range(N_CORES):
        sl = front[:, :, HSL * i:HSL * (i + 1), :]          # [B,3,12,96]
        xcm = np.ascontiguousarray(
            sl.transpose(1, 0, 2, 3).reshape(D, PIX), np.float32)
        in_maps.append(dict(common, front_cm=xcm))
    return in_maps


def _gather_output(res):
    out = np.array(res.results[0]["ff_full"], np.float32)
    for i in range(N_CORES):
        oa = res.results[i]["out_a"].reshape(D, B, HSL, W)
        out[:, :, HSL * i:HSL * (i + 1), :] += oa.transpose(1, 0, 2, 3)
    return out


def kernel(**inputs):
    import sys
    if "/opt/trn_rl_repo" not in sys.path:
        sys.path.insert(0, "/opt/trn_rl_repo")
    from concourse.bass_utils import run_bass_kernel_spmd

    in_maps = _prepare_in_maps(inputs)
    if "nc" not in _CACHED:
        _CACHED["nc"] = _build_program()
    nc = _CACHED["nc"]

    res = run_bass_kernel_spmd(nc, in_maps, core_ids=list(range(N_CORES)))
    return _gather_output(res)


# revision 56
# speedup vs baseline: 382.0642x; 1.0054x over previous
"""Trainium2 Bass kernel for nn_FKRM_85839216378385 (vq_codebook).

Strategy (8 NeuronCores, SPMD):
  - Attention branch ([B*HW, n_embed] softmax-attention over an 8192-entry
    codebook) is sharded over PIXELS: core i handles image rows
    [12*i, 12*i+12) of both batches = 2304 pixels, attending over the full
    codebook. The softmax is fused (never materialized in HBM): scores^T are
    built codebook-major ([128 codes x pix] tiles, 3 tiles per 3-bank PSUM
    buffer) with row-group-packed K=3 matmuls, exp'd on the scalar engine
    straight out of PSUM in 1536-wide calls, and contracted with
    v_ext = [v | 1] so the softmax numerator and denominator come out of one
    PSUM accumulation.
  - The program is software-pipelined around the Activation engine (the
    bottleneck: ~123us of exp work per core at 0.83ns/col): the PSF
    image-fusion branch (replicated; needs global per-batch min/max) and the
    per-chunk mlp_out tails are emitted as small filler steps between score
    tile groups so they run on DVE/Pool/PE while Activation streams exps.
  - Only Exp/Ln/Gelu activation functions are used, ordered so the table
    loads happen exactly twice (gelu set once at the head, natural_log_exp
    for everything after).
  - Weight-only transforms (k = k_w @ bg_embed etc.) are repacked on host.
"""

import numpy as np

N_CORES = 8
B, C, H, W = 2, 3, 96, 96
D = 3
NE = 8192
NWIN = 7
PAD = NWIN // 2          # 3
WP = W + 2 * PAD         # 102
HSL = H // N_CORES       # 12 rows per core (per batch)
PIX = B * HSL * W        # 2304 pixels per core
BCW = B * C * W          # 576
NVAR = float(NWIN * NWIN)          # 49
SCALE = float(D) ** -0.5
PCS = [(0, 256), (256, 512), (768, 512), (1280, 512), (1792, 512)]
NT = NE // 128           # 64 codebook tiles of 128
NRG = 3                  # k row groups (PE col-tile dst must be 0/32/64)
NKB = (NT + NRG - 1) // NRG   # 22 column blocks in the packed k
GRP = 3                  # score tiles per PSUM buffer (3 banks)


def _build_program(reps=1):
    import sys
    if "/opt/trn_rl_repo" not in sys.path:
        sys.path.insert(0, "/opt/trn_rl_repo")
    import concourse.bass as bass
    import concourse.mybir as mybir
    import concourse.tile as tile
    from concourse import bacc
    import concourse.bass_isa as bass_isa
    from contextlib import ExitStack

    f32 = mybir.dt.float32
    f32r = mybir.dt.float32r
    u32 = mybir.dt.uint32
    AF = mybir.ActivationFunctionType
    ALU = mybir.AluOpType
    AX = mybir.AxisListType
    ROP = bass_isa.ReduceOp

    nc = bacc.Bacc("TRN2", target_bir_lowering=False, debug=False,
                   num_devices=N_CORES)

    # ---------------- dram I/O ----------------
    d_front = nc.dram_tensor("front", [B, C, H, W], f32, kind="ExternalInput")
    d_back = nc.dram_tensor("back", [B, C, H, W], f32, kind="ExternalInput")
    d_xcm = nc.dram_tensor("front_cm", [D, PIX], f32r, kind="ExternalInput")
    d_k4 = nc.dram_tensor("k4", [3 * NRG, NKB * 128], f32r,
                          kind="ExternalInput")
    d_v = nc.dram_tensor("v_sb", [128, NT * 4], f32r, kind="ExternalInput")
    d_band = nc.dram_tensor("band", [H, H], f32r, kind="ExternalInput")
    # all small weight matrices packed into one [6, 30] tensor (one DMA):
    # [0:6,0:3]=w2cT [0:6,3:6]=mo2cT [0:3,6:12]=w1T [0:3,12:15]=qwT
    # [0:3,15:21]=mo1T [0:3,21:24]=fuse3T [0:3,24:27]=ones33
    # [0:1,27:30]=ones13 [0:4,30:33]=sel43 (row-3 selector)
    d_wpack = nc.dram_tensor("wpack", [6, 39], f32r, kind="ExternalInput")
    # all bias/scale vectors packed into one [6, 10] tensor (one DMA):
    # cols: b1, mob1, b2c, n1g, n1b, mob2c, n2g, n2b, fuseb, eps3
    d_vpack = nc.dram_tensor("vpack", [6, 10], f32, kind="ExternalInput")
    d_cw = nc.dram_tensor("cw", [45], f32, kind="ExternalInput")
    d_cbias = nc.dram_tensor("cbias", [4 * BCW], f32, kind="ExternalInput")

    d_oa = nc.dram_tensor("out_a", [D, PIX], f32, kind="ExternalOutput")
    d_ff = nc.dram_tensor("ff_full", [B, C, H, W], f32, kind="ExternalOutput")

    def hbcw_ap(handle, b):
        """AP over one batch of a [B,C,H,W] dram tensor ordered (h | c, w)."""
        a = handle[:, :, :, :]
        return bass.AP(tensor=a.tensor, offset=a.offset + b * C * H * W,
                       ap=[[W, H], [H * W, C], [1, W]])

    def col_ap(handle, n):
        """[n] dram vector viewed as [n, 1] (one element per partition)."""
        a = handle[:]
        return bass.AP(tensor=a.tensor, offset=a.offset, ap=[[1, n], [0, 1]])

    def bcast_ap(handle, n):
        """[n] dram vector broadcast across 96 partitions -> [96, n]."""
        a = handle[:]
        return bass.AP(tensor=a.tensor, offset=a.offset, ap=[[0, 96], [1, n]])

    with tile.TileContext(nc) as tc, ExitStack() as ctx:
        consts = ctx.enter_context(tc.tile_pool(name="consts", bufs=1))
        psf = ctx.enter_context(tc.tile_pool(name="psf", bufs=1))
        psft = ctx.enter_context(tc.tile_pool(name="psft", bufs=2))
        mlp = ctx.enter_context(tc.tile_pool(name="mlp", bufs=1))
        attn = ctx.enter_context(tc.tile_pool(name="attn", bufs=3))
        tl = ctx.enter_context(tc.tile_pool(name="tl", bufs=2))

        # ---------------- constants to SBUF ----------------
        # head-critical loads (X, wpack, vpack) lead the SP queue; the big
        # loop constants (k4, v) go on the gpsimd queue so nothing blocks
        # the mlp_in head pipeline.
        X_sb = mlp.tile([D, PIX], f32r, tag="X")
        nc.sync.dma_start(out=X_sb, in_=d_xcm[:, :])
        wp_sb = consts.tile([6, 39], f32r)
        nc.sync.dma_start(out=wp_sb, in_=d_wpack[:, :])
        vp_sb = consts.tile([6, 10], f32)
        nc.sync.dma_start(out=vp_sb, in_=d_vpack[:, :])
        w2cT_sb = wp_sb[0:6, 0:3]
        mo2cT_sb = wp_sb[0:6, 3:6]
        w1T_sb = wp_sb[0:3, 6:12]
        qwT_sb = wp_sb[0:3, 12:15]
        mo1T_sb = wp_sb[0:3, 15:21]
        fuse3_sb = wp_sb[0:3, 21:24]
        ones33_sb = wp_sb[0:3, 24:27]
        ones13_sb = wp_sb[0:1, 27:30]
        sel43_sb = wp_sb[0:4, 30:33]
        sel436_sb = wp_sb[0:4, 33:39]
        b1_sb = vp_sb[0:6, 0:1]
        mob1_sb = vp_sb[0:6, 1:2]
        b2c_sb = vp_sb[0:3, 2:3]
        n1g_sb = vp_sb[0:3, 3:4]
        n1b_sb = vp_sb[0:3, 4:5]
        mob2c_sb = vp_sb[0:3, 5:6]
        n2g_sb = vp_sb[0:3, 6:7]
        n2b_sb = vp_sb[0:3, 7:8]
        fuseb_sb = vp_sb[0:3, 8:9]
        eps3_sb = vp_sb[0:3, 9:10]
        k4_sb = consts.tile([67, NKB * 128], f32r)
        for g in range(NRG):
            nc.gpsimd.dma_start(out=k4_sb[32 * g:32 * g + 3, :],
                                in_=d_k4[3 * g:3 * g + 3, :])
        v_sb = consts.tile([128, NT * 4], f32r)
        nc.gpsimd.dma_start(out=v_sb[:, 0:NT * 2], in_=d_v[:, 0:NT * 2])
        nc.gpsimd.dma_start(out=v_sb[:, NT * 2:], in_=d_v[:, NT * 2:])
        v_sb4 = v_sb.rearrange("p (n f) -> p n f", f=4)
        band_sb = consts.tile([H, H], f32r)
        nc.gpsimd.dma_start(out=band_sb, in_=d_band[:, :])
        cw_sb = consts.tile([96, 45], f32)
        nc.gpsimd.dma_start(out=cw_sb, in_=bcast_ap(d_cw, 45))
        cbias_sb = consts.tile([96, 4 * BCW], f32)
        nc.gpsimd.dma_start(out=cbias_sb, in_=bcast_ap(d_cbias, 4 * BCW))
        # fast-inverse-sqrt magic constant, replicated for tensor_sub
        magic_sb = consts.tile([96, BCW], u32)
        nc.vector.memset(magic_sb, 0x5F3759DF)

        def emit_rsqrt(eng, dst, src, mk_tmp, newton=1):
            """dst = 1/sqrt(src) via the u32 bit trick + Newton steps.

            All ops run on `eng` (DVE or gpsimd). src/dst are f32 APs with
            identical [p, n] shape, n <= BCW, p <= 96, contiguous innermost.
            mk_tmp() yields scratch f32 APs of the same shape.
            """
            p, n = src.partition_size(), src.free_size()
            y = dst
            yu = y.bitcast(u32)
            # scalar-operand ops only lower on DVE (Pool fails the codegen
            # engine check for TensorScalar); plain muls may run on `eng`
            nc.vector.tensor_scalar(yu, src.bitcast(u32), 1, None,
                                    ALU.logical_shift_right)
            eng.tensor_sub(yu, magic_sb[0:p, 0:n], yu)
            for _ in range(newton):
                t = mk_tmp()
                eng.tensor_mul(t, y, y)
                eng.tensor_mul(t, t, src)
                nc.vector.tensor_scalar(t, t, -0.5, 1.5, ALU.mult, ALU.add)
                eng.tensor_mul(y, y, t)

        for _rep in range(reps):
            # =========================================================
            # PSF input staging (independent of everything else)
            # layout L0: [h=96 partitions | (b, c, w)]
            # =========================================================
            fr = psf.tile([96, B, C, WP], f32, tag="fr")
            bk = psf.tile([96, B, C, WP], f32, tag="bk")
            for t in (fr, bk):
                nc.vector.memset(t[:, :, :, 0:PAD], 0.0)
                nc.vector.memset(t[:, :, :, PAD + W:], 0.0)
            fb_eng = [nc.gpsimd, nc.gpsimd, nc.sync, nc.sync]
            for b in range(B):
                fb_eng[b].dma_start(out=fr[:, b, :, PAD:PAD + W],
                                    in_=hbcw_ap(d_front, b))
                fb_eng[2 + b].dma_start(out=bk[:, b, :, PAD:PAD + W],
                                        in_=hbcw_ap(d_back, b))

            # =========================================================
            # head: mlp_in + ln1 + q, chunked (channel-major [d | pix]).
            # All 5 gelus run back-to-back on Act (one gelu table load);
            # the rest of the head uses no Act at all (DVE bit-trick rsqrt
            # for the ln1 rstd). Only chunk 0 completes before the
            # attention loop; chunks 1-4 are emitted as in-loop fillers.
            # =========================================================
            q4_sb = mlp.tile([67, PIX], f32r, tag="q4")
            hsb = {}

            def make_head_steps(c, alloc):
                """Three filler steps computing q4 for chunk c."""
                off, n = PCS[c]
                st = {}

                def mk_tmp():
                    return tl.tile([6, 512], f32, tag="t_tmp", bufs=3,
                                   name=f"nt_{c}")[0:3, 0:n]

                def hd1():
                    yp = alloc(3, 512, f"yps_{c}")
                    nc.tensor.matmul(yp[:, 0:n], w2cT_sb, hsb[c][:, 0:n],
                                     start=True, stop=True)
                    y_t = tl.tile([3, 512], f32, tag="y", name=f"y_{c}")
                    nc.vector.tensor_scalar(y_t[:, 0:n], yp[:, 0:n],
                                            b2c_sb, None, ALU.add)
                    sq_t = tl.tile([6, 512], f32r, tag="t_tmp", bufs=3,
                                   name=f"sq_{c}")[0:3, :]
                    nc.vector.tensor_mul(sq_t[:, 0:n], y_t[:, 0:n],
                                         y_t[:, 0:n])
                    st["y"], st["sq"] = y_t, sq_t

                def hd2():
                    vp = alloc(3, 512, f"vps_{c}")
                    nc.tensor.matmul(vp[:, 0:n], ones33_sb,
                                     st["sq"][:, 0:n], start=True, stop=True)
                    u_t = tl.tile([6, 512], f32, tag="t_tmp", bufs=3,
                                  name=f"u_{c}")[0:3, :]
                    nc.vector.tensor_scalar_add(u_t[:, 0:n], vp[:, 0:n], 1e-5)
                    r_t = tl.tile([3, 512], f32, tag="y", name=f"rst_{c}")
                    emit_rsqrt(nc.vector, r_t[:, 0:n], u_t[:, 0:n], mk_tmp,
                               newton=1)
                    st["r"] = r_t

                def hd3():
                    x2_t = tl.tile([6, 512], f32r, tag="t_tmp", bufs=3,
                                   name=f"x2_{c}")[0:3, :]
                    nc.vector.tensor_mul(x2_t[:, 0:n], st["y"][:, 0:n],
                                         st["r"][:, 0:n])
                    nc.vector.tensor_scalar(x2_t[:, 0:n], x2_t[:, 0:n],
                                            n1g_sb, n1b_sb, ALU.mult, ALU.add)
                    qp = alloc(3, 512, f"qps_{c}")
                    nc.tensor.matmul(qp[:, 0:n], qwT_sb, x2_t[:, 0:n],
                                     start=True, stop=True)
                    # replicate q to the three k row groups (PE column
                    # tiling is rejected by codegen; DVE writes to 32-aligned
                    # partition bases are fine)
                    for g in range(NRG):
                        nc.vector.tensor_copy(
                            q4_sb[32 * g:32 * g + 3, off:off + n], qp[:, 0:n])

                return [hd1, hd2, hd3]

            with tc.tile_pool(name="ps_head", bufs=1, space="PSUM") as psH:
                hps = {}

                # preload the gelu table before any data-dependent act call
                dumg = tl.tile([1, 8], f32, tag="dume", bufs=1, name="dumg")
                nc.scalar.activation(dumg[:, 0:1], vp_sb[0:1, 0:1],
                                     AF.Gelu_apprx_tanh)

                def emit_gelu(c):
                    off, n = PCS[c]
                    # bufs=5: h chunks 1-4 stay live until their in-loop
                    # y-matmul fillers
                    ht = tl.tile([6, 512], f32r, tag="hsb", bufs=5,
                                 name=f"h_{c}")
                    nc.scalar.activation(ht[:, 0:n], hps[c][:, 0:n],
                                         AF.Gelu_apprx_tanh, bias=b1_sb)
                    hsb[c] = ht

                for c, (off, n) in enumerate(PCS):
                    hp = psH.tile([6, 512], f32, tag="hps", bufs=2,
                                  name=f"hps_{c}")
                    nc.tensor.matmul(hp[:, 0:n], w1T_sb, X_sb[:, off:off + n],
                                     start=True, stop=True)
                    hps[c] = hp
                    if c >= 1:
                        emit_gelu(c - 1)
                emit_gelu(len(PCS) - 1)

                # preload the exp table while the head pipeline runs, so
                # the first attention exp doesn't pay the 1.28us table load
                dume = tl.tile([1, 8], f32, tag="dume", bufs=1)
                nc.scalar.activation(dume[:, 0:1], vp_sb[0:1, 0:1], AF.Exp)

                def psh_alloc(p, w, name):
                    return psH.tile([p, w], f32, tag="hd", bufs=2, name=name)

                for fn in make_head_steps(0, psh_alloc):
                    fn()

            # =========================================================
            # PSF stage A: squares (Pool) + 7-tap horizontal box (DVE)
            # =========================================================
            sqf = psf.tile([96, B, C, WP], f32, tag="sqf")
            sqb = psf.tile([96, B, C, WP], f32, tag="sqb")
            nc.gpsimd.tensor_mul(sqf, fr, fr)
            nc.gpsimd.tensor_mul(sqb, bk, bk)

            def hbox_half(dst, src, first):
                """7-tap horizontal box sum, split in two filler halves."""
                def fn():
                    if first:
                        nc.vector.tensor_add(dst, src[:, :, :, 0:W],
                                             src[:, :, :, 1:1 + W])
                        for j in range(2, 4):
                            nc.vector.tensor_add(dst, dst,
                                                 src[:, :, :, j:j + W])
                    else:
                        for j in range(4, NWIN):
                            nc.vector.tensor_add(dst, dst,
                                                 src[:, :, :, j:j + W])
                return fn

            brh_f = psf.tile([96, B, C, W], f32r, tag="brh_f")
            brh_f2 = psf.tile([96, B, C, W], f32r, tag="brh_f2")
            brh_b = psf.tile([96, B, C, W], f32r, tag="brh_b")
            brh_b2 = psf.tile([96, B, C, W], f32r, tag="brh_b2")

            # PSF result tiles (filled by filler steps inside the loop;
            # vbox overwrites brh_* in place, so s1f==brh_f etc.)
            m_f = psf.tile([96, B, C, W], f32, tag="m_f")
            r_f = psf.tile([96, B, C, W], f32, tag="r_f")
            m_b = psf.tile([96, B, C, W], f32, tag="m_b")
            r_b = psf.tile([96, B, C, W], f32, tag="r_b")
            sd_b = psf.tile([96, B, C, W], f32, tag="sd_b")
            xnf = psf.tile([96, B, C, W], f32, tag="xnf")
            xnb = psf.tile([96, B, C, W], f32, tag="xnb")
            xad = psf.tile([96, B, C, W], f32, tag="xad")
            EE = psf.tile([96, B, C, W], f32, tag="EE")
            FF = psf.tile([96, B, C, W], f32, tag="FF")
            GG = psf.tile([96, B, C, W], f32, tag="GG")
            HH = psf.tile([96, B, C, W], f32, tag="HH")
            dot = psf.tile([96, B, W], f32, tag="dot")
            f2 = psf.tile([96, B, W], f32, tag="f2")
            g2 = psf.tile([96, B, W], f32, tag="g2")
            S = psf.tile([96, B, W], f32, tag="S")
            Sn = psf.tile([96, B, W], f32, tag="Sn")

            # =========================================================
            # attention main loop, with PSF + mlp_out tails as fillers
            # =========================================================
            with tc.tile_pool(name="ps_sc", bufs=2, space="PSUM") as ps_sc, \
                 tc.tile_pool(name="ps_num", bufs=1, space="PSUM") as ps_num, \
                 tc.tile_pool(name="ps_aux", bufs=1, space="PSUM") as ps_aux:

                # ---------- filler step definitions ----------
                def aux_tile(p, w, name):
                    return ps_aux.tile([p, w], f32, tag="aux", name=name)

                def vbox_step(srcdst, idx):
                    """srcdst = Band^T @ srcdst over the h (partition) axis,
                    in place (each 288-col half is matmul'd then written
                    back)."""
                    def fn():
                        sflat = srcdst.rearrange("p b c w -> p (b c w)")
                        for half in range(2):
                            slh = slice(half * 288, half * 288 + 288)
                            bp = aux_tile(96, 288, f"vb_{idx}_{half}")
                            nc.tensor.matmul(bp, band_sb, sflat[:, slh],
                                             start=True, stop=True)
                            nc.vector.tensor_copy(sflat[:, slh], bp)
                    return fn

                def stats1_step(s1, s2, tagp):
                    def fn():
                        v_t = psft.tile([96, B, C, W], f32, tag="st_v",
                                        bufs=1, name=f"v_{tagp}")
                        u_t = psft.tile([96, B, C, W], f32, tag="st_u",
                                        bufs=1, name=f"u_{tagp}")
                        nc.vector.tensor_mul(v_t, s1, s1)
                        nc.vector.tensor_scalar_mul(
                            v_t, v_t, -1.0 / (NVAR * (NVAR - 1)))
                        nc.vector.tensor_scalar_mul(u_t, s2, 1.0 / (NVAR - 1))
                        nc.vector.tensor_add(v_t, v_t, u_t)
                        self_d[tagp] = v_t
                    return fn

                self_d = {}

                def stats2_step(s1, mean_t, rstd_t, sd_t, tagp):
                    def fn():
                        # rstd = rsqrt(var) on DVE (no act-table traffic);
                        # sd (needed only for 'back') = var * rstd
                        v_t = self_d[tagp]
                        vfl = v_t.rearrange("p b c w -> p (b c w)")
                        rfl = rstd_t.rearrange("p b c w -> p (b c w)")

                        def mk_tmp():
                            t = psft.tile([96, B, C, W], f32, tag="st_r",
                                          bufs=1, name=f"rs_{tagp}")
                            return t.rearrange("p b c w -> p (b c w)")

                        emit_rsqrt(nc.vector, rfl, vfl, mk_tmp, newton=1)
                        if sd_t is not None:
                            nc.vector.tensor_mul(sd_t, v_t, rstd_t)
                        nc.vector.tensor_scalar_mul(mean_t, s1, 1.0 / NVAR)
                    return fn

                def mvn_step(dst, src, m_t, r_t):
                    def fn():
                        nc.vector.tensor_sub(dst, src[:, :, :, PAD:PAD + W], m_t)
                        nc.vector.tensor_mul(dst, dst, r_t)
                    return fn

                def adain_step():
                    nc.vector.tensor_mul(xad, xnf, sd_b)
                    nc.vector.tensor_add(xad, xad, m_b)

                def conv3(dst, src, wbase, bias_idx, name):
                    def fn():
                        for co in range(3):
                            dco = dst[:, :, co, :]
                            t = psft.tile([96, B, W], f32, tag="conv_t",
                                          name=f"cv_{name}_{co}")
                            nc.vector.tensor_scalar_mul(
                                dco, src[:, :, 0, :],
                                cw_sb[:, wbase + co * 3:wbase + co * 3 + 1])
                            nc.vector.tensor_scalar_mul(
                                t, src[:, :, 1, :],
                                cw_sb[:, wbase + co * 3 + 1:wbase + co * 3 + 2])
                            nc.vector.tensor_add(dco, dco, t)
                            nc.vector.tensor_scalar_mul(
                                t, src[:, :, 2, :],
                                cw_sb[:, wbase + co * 3 + 2:wbase + co * 3 + 3])
                            nc.vector.tensor_add(dco, dco, t)
                        if bias_idx is not None:
                            dflat = dst.rearrange("p b c w -> p (b c w)")
                            nc.vector.tensor_add(
                                dflat, dflat,
                                cbias_sb[:, bias_idx * BCW:(bias_idx + 1) * BCW])
                    return fn

                def cos1_step():
                    tmc = psft.tile([96, B, W], f32, tag="cos_t", name="tmc1")
                    nc.vector.tensor_mul(dot, FF[:, :, 0, :], GG[:, :, 0, :])
                    nc.vector.tensor_mul(f2, FF[:, :, 0, :], FF[:, :, 0, :])
                    nc.vector.tensor_mul(g2, GG[:, :, 0, :], GG[:, :, 0, :])
                    nc.vector.tensor_mul(tmc, FF[:, :, 1, :], GG[:, :, 1, :])
                    nc.vector.tensor_add(dot, dot, tmc)

                def cos2_step():
                    tmc = psft.tile([96, B, W], f32, tag="cos_t", name="tmc2")
                    nc.vector.tensor_mul(tmc, FF[:, :, 1, :], FF[:, :, 1, :])
                    nc.vector.tensor_add(f2, f2, tmc)
                    nc.vector.tensor_mul(tmc, GG[:, :, 1, :], GG[:, :, 1, :])
                    nc.vector.tensor_add(g2, g2, tmc)
                    nc.vector.tensor_mul(tmc, FF[:, :, 2, :], GG[:, :, 2, :])
                    nc.vector.tensor_add(dot, dot, tmc)

                def cos3_step():
                    tmc = psft.tile([96, B, W], f32, tag="cos_t", name="tmc3")
                    nc.vector.tensor_mul(tmc, FF[:, :, 2, :], FF[:, :, 2, :])
                    nc.vector.tensor_add(f2, f2, tmc)
                    nc.vector.tensor_mul(tmc, GG[:, :, 2, :], GG[:, :, 2, :])
                    nc.vector.tensor_add(g2, g2, tmc)
                    nc.vector.tensor_mul(f2, f2, g2)          # F2*G2
                    # 1/(Fn*Gn) = rsqrt(F2*G2) on DVE
                    f2fl = f2.rearrange("p b w -> p (b w)")
                    g2fl = g2.rearrange("p b w -> p (b w)")

                    def mk_tmp():
                        t = psft.tile([96, B, W], f32, tag="cos_t",
                                      name="tmc4")
                        return t.rearrange("p b w -> p (b w)")

                    emit_rsqrt(nc.vector, g2fl, f2fl, mk_tmp, newton=1)
                    nc.vector.tensor_mul(S, dot, g2)

                mm_d = {}

                def minmax1_step():
                    rmx = psft.tile([96, 2], f32, tag="rmx", name="rmx")
                    rmn = psft.tile([96, 2], f32, tag="rmn", name="rmn")
                    nS = psft.tile([96, B, W], f32, tag="nS", name="nS")
                    nc.vector.tensor_scalar_mul(nS, S, -1.0)
                    nc.vector.tensor_reduce(rmx, S, axis=AX.X, op=ALU.max)
                    nc.vector.tensor_reduce(rmn, nS, axis=AX.X, op=ALU.max)
                    mm_d["rmx"], mm_d["rmn"] = rmx, rmn

                def minmax2_step():
                    nmn_bc = psft.tile([96, 2], f32, tag="nmn_bc", name="nmn_bc")
                    mx_bc = psft.tile([96, 2], f32, tag="mx_bc", name="mx_bc")
                    dd_bc = psft.tile([96, 2], f32, tag="dd_bc", name="dd_bc")
                    ri_bc = psft.tile([96, 2], f32, tag="ri_bc", name="ri_bc")
                    nc.gpsimd.partition_all_reduce(mx_bc, mm_d["rmx"], 96, ROP.max)
                    nc.gpsimd.partition_all_reduce(nmn_bc, mm_d["rmn"], 96, ROP.max)
                    nc.vector.tensor_add(dd_bc, mx_bc, nmn_bc)    # max - min
                    nc.vector.reciprocal(ri_bc, dd_bc)
                    mm_d["nmn"], mm_d["ri"] = nmn_bc, ri_bc

                def minmax3_step():
                    for b in range(B):
                        nc.vector.tensor_scalar(
                            Sn[:, b, :], S[:, b, :],
                            mm_d["nmn"][:, b:b + 1], mm_d["ri"][:, b:b + 1],
                            ALU.add, ALU.mult)

                def fuse1_step():
                    nc.vector.tensor_sub(xnf, EE, HH)   # reuse xnf as scratch
                    for cc in range(3):
                        nc.vector.tensor_mul(xnf[:, :, cc, :],
                                             xnf[:, :, cc, :], Sn)

                def fuse2_step():
                    ff_flat = xnf.rearrange("p b c w -> p (b c w)")
                    hh_flat = HH.rearrange("p b c w -> p (b c w)")
                    nc.vector.tensor_add(ff_flat, ff_flat, hh_flat)

                def ffdma_step():
                    for b in range(B):
                        nc.sync.dma_start(out=hbcw_ap(d_ff, b),
                                          in_=xnb[:, b, :, :])

                # ---------- mlp_out tail steps for one chunk ----------
                # transient [<=6, 512] scratch shares one 3-deep rotating tag
                tail_d = {}

                def tmp_tile(c, nm):
                    return tl.tile([6, 512], f32, tag="t_tmp", bufs=3,
                                   name=f"{nm}_{c}")

                def t0_step(c):
                    def fn():
                        off, n = PCS[c]
                        h2p = aux_tile(6, 512, f"h2p_{c}")
                        nc.tensor.matmul(h2p[:, 0:n], mo1T_sb,
                                         tail_d[c]["att"][:, 0:n],
                                         start=True, stop=True)
                        xg = tl.tile([6, 512], f32, tag="t_xg", name=f"xg_{c}")
                        nc.vector.tensor_scalar(xg[:, 0:n], h2p[:, 0:n],
                                                mob1_sb, None, ALU.add)
                        tail_d[c]["xg"] = xg
                    return fn

                def t1_step(c):
                    def fn():
                        off, n = PCS[c]
                        d = tail_d[c]
                        eng = nc.gpsimd if c < len(PCS) - 1 else nc.vector
                        s = tmp_tile(c, "s")
                        eng.tensor_mul(s[:, 0:n], d["xg"][:, 0:n],
                                       d["xg"][:, 0:n])
                        eng.tensor_mul(s[:, 0:n], s[:, 0:n],
                                       d["xg"][:, 0:n])
                        d["cb"] = s
                    return fn

                def t2_step(c):
                    def fn():
                        off, n = PCS[c]
                        d = tail_d[c]
                        t = tmp_tile(c, "t")
                        nc.vector.tensor_scalar_mul(t[:, 0:n], d["cb"][:, 0:n],
                                                    0.044715)
                        nc.vector.tensor_add(t[:, 0:n], t[:, 0:n],
                                             d["xg"][:, 0:n])
                        e = tmp_tile(c, "e")
                        nc.scalar.activation(e[:, 0:n], t[:, 0:n], AF.Exp,
                                             scale=1.5957691216057308)
                        d["e"] = e
                    return fn

                def t3_step(c):
                    def fn():
                        off, n = PCS[c]
                        d = tail_d[c]
                        dd = tmp_tile(c, "dd")
                        nc.vector.tensor_scalar_add(dd[:, 0:n], d["e"][:, 0:n],
                                                    1.0)
                        r = tmp_tile(c, "r")
                        nc.vector.reciprocal(r[:, 0:n], dd[:, 0:n])
                        nc.vector.tensor_mul(r[:, 0:n], d["e"][:, 0:n],
                                             r[:, 0:n])
                        h2 = tl.tile([6, 512], f32r, tag="t_h2", name=f"h2_{c}")
                        nc.vector.tensor_mul(h2[:, 0:n], d["xg"][:, 0:n],
                                             r[:, 0:n])
                        d["h2"] = h2
                    return fn

                def t4_step(c):
                    def fn():
                        off, n = PCS[c]
                        d = tail_d[c]
                        y2p = aux_tile(3, 512, f"y2p_{c}")
                        nc.tensor.matmul(y2p[:, 0:n], mo2cT_sb,
                                         d["h2"][:, 0:n], start=True, stop=True)
                        y2 = tl.tile([3, 512], f32, tag="t_y2", name=f"ty2_{c}")
                        nc.vector.tensor_scalar(y2[:, 0:n], y2p[:, 0:n],
                                                mob2c_sb, None, ALU.add)
                        sq2 = tl.tile([3, 512], f32r, tag="t_sq2",
                                      name=f"tsq_{c}")
                        nc.vector.tensor_mul(sq2[:, 0:n], y2[:, 0:n],
                                             y2[:, 0:n])
                        d["y2"], d["sq2"] = y2, sq2
                    return fn

                def t5_step(c):
                    # rstd for ln2 = rsqrt(var + eps). Pipelined chunks use
                    # the gpsimd/DVE bit-trick rsqrt (keeps Act free for the
                    # exp stream); the LAST chunk uses Act Ln/Exp — the act
                    # engine is idle by then and its serial chain is shorter
                    # than the 7-op DVE Newton chain.
                    def fn():
                        off, n = PCS[c]
                        d = tail_d[c]
                        v2p = aux_tile(3, 512, f"v2p_{c}")
                        nc.tensor.matmul(v2p[:, 0:n], ones33_sb,
                                         d["sq2"][:, 0:n], start=True, stop=True)
                        r32 = tl.tile([3, 512], f32, tag="t_r32",
                                      name=f"tr32_{c}")
                        if c == len(PCS) - 1:
                            l2 = tmp_tile(c, "l2")
                            nc.scalar.activation(l2[0:3, 0:n], v2p[:, 0:n],
                                                 AF.Ln, bias=eps3_sb)
                            nc.scalar.activation(r32[:, 0:n], l2[0:3, 0:n],
                                                 AF.Exp, scale=-0.5)
                        else:
                            u2 = tmp_tile(c, "u2")
                            # gpsimd cannot read PSUM; eps-add stays on DVE
                            nc.vector.tensor_scalar_add(u2[0:3, 0:n],
                                                        v2p[:, 0:n], 1e-5)

                            def mk_tmp():
                                return tmp_tile(c, "rt")[0:3, 0:n]

                            emit_rsqrt(nc.gpsimd, r32[:, 0:n], u2[0:3, 0:n],
                                       mk_tmp, newton=1)
                        d["r32"] = r32
                    return fn

                def t6_step(c):
                    # ln2's gamma/beta are folded into fuse3/fuseb host-side
                    def fn():
                        off, n = PCS[c]
                        d = tail_d[c]
                        x5 = tl.tile([3, 512], f32r, tag="t_x5", bufs=3,
                                     name=f"x5_{c}")
                        nc.vector.tensor_mul(x5[:, 0:n], d["y2"][:, 0:n],
                                             d["r32"][:, 0:n])
                        d["x5"] = x5
                    return fn

                def t7_step(c):
                    def fn():
                        off, n = PCS[c]
                        d = tail_d[c]
                        op = aux_tile(3, 512, f"op_{c}")
                        nc.tensor.matmul(op[:, 0:n], fuse3_sb, d["x5"][:, 0:n],
                                         start=True, stop=True)
                        oa = tl.tile([3, 512], f32, tag="t_x5", bufs=3,
                                    name=f"oa_{c}")
                        nc.vector.tensor_scalar(oa[:, 0:n], op[:, 0:n],
                                                fuseb_sb, None, ALU.add)
                        nc.sync.dma_start(out=d_oa[:, off:off + n],
                                          in_=oa[:, 0:n])
                    return fn

                def t0g_step(c):
                    # last chunk only: mo1 runs on the UN-normalized softmax
                    # numerator (legal: the softmax denominator is a
                    # per-column scale that commutes with the channel
                    # matmul), the division lands on the matmul output, and
                    # gelu uses the act table (Act is idle once the loop exps
                    # end). Shortens the end-of-program chain.
                    def fn():
                        off, n = PCS[c]
                        num_sb = ep_d[c]
                        h2p = aux_tile(6, 512, f"h2p_{c}")
                        nc.tensor.matmul(h2p[:, 0:n], mo1T_sb,
                                         num_sb[0:3, 0:n],
                                         start=True, stop=True)
                        # the num bank is free after the copy; using it for
                        # d6p avoids a WAR cycle on the single aux slot
                        d6p = ps_num.tile([6, 512], f32, tag="num",
                                          name=f"d6p_{c}")
                        nc.tensor.matmul(d6p[:, 0:n], sel436_sb,
                                         num_sb[:, 0:n],
                                         start=True, stop=True)
                        r6 = tl.tile([6, 512], f32, tag="t_r6", bufs=1,
                                     name=f"r6_{c}")
                        nc.vector.reciprocal(r6[:, 0:n], d6p[:, 0:n])
                        xg = tl.tile([6, 512], f32, tag="t_xg",
                                     name=f"xg_{c}")
                        nc.vector.tensor_mul(xg[:, 0:n], h2p[:, 0:n],
                                             r6[:, 0:n])
                        h2 = tl.tile([6, 512], f32r, tag="t_h2",
                                     name=f"h2_{c}")
                        nc.scalar.activation(h2[:, 0:n], xg[:, 0:n],
                                             AF.Gelu_apprx_tanh, bias=mob1_sb)
                        tail_d[c] = {"h2": h2}
                    return fn

                def tail_steps(c):
                    if c == len(PCS) - 1:
                        return [t0g_step(c), t4_step(c), t5_step(c),
                                t6_step(c), t7_step(c)]
                    return [t0_step(c), t1_step(c), t2_step(c), t3_step(c),
                            t4_step(c), t5_step(c), t6_step(c), t7_step(c)]

                # nt=0 pre-tile: scored into the aux bank and exp'd ahead
                # of its chunk
                ex0_d = {}

                def pre_step(c):
                    def fn():
                        off, n = PCS[c]
                        scp = aux_tile(128, 512, f"scp_{c}")
                        nc.tensor.matmul(scp[:, 0:n], k4_sb[0:3, 0:128],
                                         q4_sb[0:3, off:off + n],
                                         tile_position=(0, 0),
                                         start=True, stop=True)
                        ex0 = tl.tile([128, 512], f32r, tag="ex0", bufs=2,
                                      name=f"ex0_{c}")
                        nc.scalar.activation(ex0[:, 0:n], scp[:, 0:n], AF.Exp)
                        ex0_d[c] = ex0
                    return fn

                # ---------- filler schedule: slot (chunk, group) -> fns ----
                slot = {}

                def put(c, gi, fn):
                    slot.setdefault((c, gi), []).append(fn)

                # head chunks 1-4: hd1/hd2 early in chunk c-1; hd3 at
                # gi 9 so its q-matmuls sit well after the rsqrt chain
                for hc in range(1, 5):
                    h1f, h2f, h3f = make_head_steps(hc, aux_tile)
                    put(hc - 1, 1, h1f)
                    put(hc - 1, 3, h2f)
                    put(hc - 1, 9, h3f)

                # mlp_out tails: t2's act-exp and t6 get extra slack so they
                # never block the in-order Act/PE streams
                TSLOTS = [4, 6, 8, 10, 12, 14, 19, 20]
                for tc_ in range(4):
                    for i, fn in enumerate(tail_steps(tc_)):
                        put(tc_ + 1, TSLOTS[i], fn)

                # next chunk's nt=0 pre-tile (aux PSUM) near this chunk's end
                for pc in range(1, 5):
                    put(pc - 1, 16, pre_step(pc))

                # PSF pipeline, odd slots
                put(0, 5, hbox_half(brh_f, fr, True))
                put(0, 7, hbox_half(brh_f, fr, False))
                put(0, 11, hbox_half(brh_f2, sqf, True))
                put(0, 13, hbox_half(brh_f2, sqf, False))
                put(0, 15, hbox_half(brh_b, bk, True))
                put(0, 17, hbox_half(brh_b, bk, False))
                put(0, 19, hbox_half(brh_b2, sqb, True))
                put(1, 5, hbox_half(brh_b2, sqb, False))
                put(1, 7, vbox_step(brh_f, 0))
                put(1, 11, vbox_step(brh_f2, 1))
                put(1, 13, vbox_step(brh_b, 2))
                put(1, 15, vbox_step(brh_b2, 3))
                put(1, 17, stats1_step(brh_f, brh_f2, "f"))
                put(1, 19, stats2_step(brh_f, m_f, r_f, None, "f"))
                put(2, 5, stats1_step(brh_b, brh_b2, "b"))
                put(2, 7, stats2_step(brh_b, m_b, r_b, sd_b, "b"))
                put(2, 11, mvn_step(xnf, fr, m_f, r_f))
                put(2, 13, mvn_step(xnb, bk, m_b, r_b))
                put(2, 15, adain_step)
                put(2, 17, conv3(EE, xad, 0, 0, "EE"))
                put(2, 19, conv3(FF, xnf, 9, 1, "FF"))
                put(3, 5, conv3(GG, xnb, 18, 2, "GG"))
                put(3, 7, conv3(HH, bk[:, :, :, PAD:PAD + W], 27, 3, "HH"))
                put(3, 11, cos1_step)
                put(3, 13, cos2_step)
                put(3, 15, cos3_step)
                put(3, 17, minmax1_step)
                put(3, 19, minmax2_step)
                put(4, 5, minmax3_step)
                put(4, 7, fuse1_step)
                put(4, 11, fuse2_step)
                put(4, 13, conv3(xnb, xnf, 36, None, "ffo"))  # ffo into xnb
                put(4, 15, ffdma_step)

                pre_step(0)()

                # ---------- the loop ----------
                # tile nt=0 of each chunk is peeled into the spare aux PSUM
                # bank and emitted during the previous chunk; the remaining
                # 63 tiles form exactly 21 groups of 3. Contractions lag one
                # group behind their scores in the PE stream, and a chunk's
                # LAST contraction (plus the whole epilogue) is flushed after
                # the NEXT chunk's second score group — the in-order PE
                # stream then never waits on the exp stream at a boundary.
                groups = [list(range(s, s + GRP)) for s in range(1, NT, GRP)]

                def contract(num_ps, ex, nts, n, last):
                    for u, nt in enumerate(nts):
                        nc.tensor.matmul(
                            num_ps[:, 0:n], v_sb4[:, nt, :],
                            ex[:, u * n:(u + 1) * n],
                            start=False, stop=(nt == NT - 1))
                    if last:
                        # epilogue part 1: free the num bank
                        c_, n_, nps = last
                        num_sb = attn.tile([4, 512], f32r, tag="numsb",
                                           bufs=2, name=f"numsb_{c_}")
                        nc.vector.tensor_copy(num_sb[:, 0:n_], nps[:, 0:n_])
                        ep_d[c_] = num_sb

                def ep2_step(c):
                    def fn():
                        off, n = PCS[c]
                        num_sb = ep_d[c]
                        d3p = aux_tile(3, 512, f"d3p_{c}")
                        nc.tensor.matmul(d3p[:, 0:n], sel43_sb,
                                         num_sb[:, 0:n],
                                         start=True, stop=True)
                        r3i = attn.tile([3, 512], f32, tag="r3i", bufs=2,
                                        name=f"r3i_{c}")
                        nc.vector.reciprocal(r3i[:, 0:n], d3p[:, 0:n])
                        att_t = tl.tile([3, 512], f32r, tag="att",
                                        name=f"att_{c}")
                        nc.vector.tensor_mul(att_t[:, 0:n], num_sb[0:3, 0:n],
                                             r3i[:, 0:n])
                        tail_d[c] = {"att": att_t}
                    return fn

                ep_d = {}
                held = None   # (num_ps, ex, nts, n, last_info) from prev grp

                for c, (off, n) in enumerate(PCS):
                    sl = slice(off, off + n)
                    num_ps = ps_num.tile([4, 512], f32, tag="num",
                                         name=f"num_{c}")
                    for gi, nts in enumerate(groups):
                        w = len(nts) * 512
                        sc = ps_sc.tile([128, w], f32, tag="sc",
                                        name=f"sc_{c}_{gi}")
                        for u, nt in enumerate(nts):
                            r, m = nt % NRG, nt // NRG
                            nc.tensor.matmul(
                                sc[:, u * 512:u * 512 + n],
                                k4_sb[32 * r:32 * r + 3, 128 * m:128 * m + 128],
                                q4_sb[32 * r:32 * r + 3, sl],
                                tile_position=(32 * r, 0),
                                start=True, stop=True)
                        ex = attn.tile([128, GRP * 512], f32r, tag="ex",
                                       bufs=3, name=f"ex_{c}_{gi}")
                        sc_v = sc.rearrange("p (g c) -> p g c",
                                            g=len(nts))[:, :, 0:n]
                        nc.scalar.activation(ex[:, 0:len(nts) * n], sc_v,
                                             AF.Exp)
                        if gi == 1:
                            if held is not None:
                                contract(*held)
                                held = None
                            # this chunk's accumulation starts with the
                            # pre-tile (start=True resets PSUM)
                            nc.tensor.matmul(num_ps[:, 0:n], v_sb4[:, 0, :],
                                             ex0_d[c][:, 0:n],
                                             start=True, stop=False)
                            contract(num_ps, exs[0], groups[0], n, None)
                        elif gi >= 2:
                            contract(num_ps, exs[gi - 1], groups[gi - 1], n,
                                     None)
                        if gi == 0:
                            exs = {}
                        exs[gi] = ex
                        for fn in slot.get((c, gi), ()):
                            fn()
                    held = (num_ps, exs[len(groups) - 1],
                            groups[len(groups) - 1], n, (c, n, num_ps))
                    if c < len(PCS) - 1:
                        put(c + 1, 2, ep2_step(c))

                # flush the last chunk's contraction; its tail consumes
                # ep_d directly (no att materialization needed)
                contract(*held)
                held = None

                # tail for the last chunk
                for fn in tail_steps(4):
                    fn()

    nc.compile()
    return nc


_CACHED = {}


def _prepare_in_maps(inputs):
    f = lambda k: np.asarray(inputs[k], np.float32)
    front, back = f("front"), f("back")
    bg = f("bg_embed")                      # [3, 8192]
    q_w, k_w, v_w = f("q_w"), f("k_w"), f("v_w")
    mi_w1, mi_b1 = f("mi_w1"), f("mi_b1")
    mi_w2, mi_b2 = f("mi_w2"), f("mi_b2")
    mo_w1, mo_b1 = f("mo_w1"), f("mo_b1")
    mo_w2, mo_b2 = f("mo_w2"), f("mo_b2")
    n1_g, n1_b, n2_g, n2_b = f("n1_g"), f("n1_b"), f("n2_g"), f("n2_b")
    e_w, e_b = f("e_w"), f("e_b")
    f_w, f_b = f("f_w"), f("f_b")
    g_w, g_b = f("g_w"), f("g_b")
    h_w, h_b = f("h_w"), f("h_b")
    fuse_w, fuse_b = f("fuse_w"), f("fuse_b")

    # ---- host-side weight repacking (tiny, O(n_embed * d)) ----
    kT = (k_w @ bg) * SCALE                                   # [3, NE]
    # row-group-packed k: tile nt -> row group r = nt % 3, col block nt // 3
    NRG, NKB = 3, (NT + 2) // 3
    k4 = np.zeros((3 * NRG, NKB * 128), np.float32)
    for nt in range(NT):
        r, m = nt % NRG, nt // NRG
        k4[3 * r:3 * r + 3, 128 * m:128 * (m + 1)] = \
            kT[:, nt * 128:(nt + 1) * 128]
    v = bg.T @ v_w.T                                          # [NE, 3]
    v_ext = np.concatenate([v, np.ones((NE, 1), np.float32)], 1)
    v_np = np.ascontiguousarray(
        v_ext.reshape(NT, 128, 4).transpose(1, 0, 2).reshape(128, NT * 4))
    hh, ww = np.meshgrid(np.arange(H), np.arange(H), indexing="ij")
    band = (np.abs(hh - ww) <= PAD).astype(np.float32)
    w2c = mi_w2 - mi_w2.mean(0, keepdims=True)
    b2c = mi_b2 - mi_b2.mean()
    mo2c = mo_w2 - mo_w2.mean(0, keepdims=True)
    mob2c = mo_b2 - mo_b2.mean()
    cw = np.concatenate([e_w.ravel(), f_w.ravel(), g_w.ravel(),
                         h_w.ravel(), fuse_w[:, 3:6].ravel()])
    cbias = np.concatenate(
        [np.tile(np.repeat(bb, W), B) for bb in (e_b, f_b, g_b, h_b)])

    wpack = np.zeros((6, 39), np.float32)
    wpack[0:6, 0:3] = w2c.T
    wpack[0:6, 3:6] = mo2c.T
    wpack[0:3, 6:12] = mi_w1.T
    wpack[0:3, 12:15] = q_w.T
    wpack[0:3, 15:21] = mo_w1.T
    wpack[0:3, 21:24] = fuse_w[:, 0:3].T * n2_g[:, None]
    wpack[0:3, 24:27] = 1.0 / 3.0
    wpack[0:1, 27:30] = 1.0
    wpack[3, 30:39] = 1.0
    vpack = np.zeros((6, 10), np.float32)
    vpack[0:6, 0] = mi_b1
    vpack[0:6, 1] = mo_b1
    vpack[0:3, 2] = b2c
    vpack[0:3, 3] = n1_g
    vpack[0:3, 4] = n1_b
    vpack[0:3, 5] = mob2c
    vpack[0:3, 6] = n2_g
    vpack[0:3, 7] = n2_b
    vpack[0:3, 8] = fuse_b + fuse_w[:, 0:3] @ n2_b
    vpack[0:3, 9] = 1e-5

    common = dict(
        front=front, back=back,
        k4=k4, v_sb=v_np,
        band=band,
        wpack=wpack, vpack=vpack,
        cw=np.ascontiguousarray(cw, np.float32),
        cbias=np.ascontiguousarray(cbias, np.float32),
    )
    common = {k: np.ascontiguousarray(v2, np.float32)
              for k, v2 in common.items()}

    in_maps = []
    for i in range(N_CORES):
        sl = front[:, :, HSL * i:HSL * (i + 1), :]          # [B,3,12,96]
        xcm = np.ascontiguousarray(
            sl.transpose(1, 0, 2, 3).reshape(D, PIX), np.float32)
        in_maps.append(dict(common, front_cm=xcm))
    return in_maps


def _gather_output(res):
    out = np.array(res.results[0]["ff_full"], np.float32)
    for i in range(N_CORES):
        oa = res.results[i]["out_a"].reshape(D, B, HSL, W)
        out[:, :, HSL * i:HSL * (i + 1), :] += oa.transpose(1, 0, 2, 3)
    return out


def kernel(**inputs):
    import sys
    if "/opt/trn_rl_repo" not in sys.path:
        sys.path.insert(0, "/opt/trn_rl_repo")
    from concourse.bass_utils import run_bass_kernel_spmd

    in_maps = _prepare_in_maps(inputs)
    if "nc" not in _CACHED:
        _CACHED["nc"] = _build_program()
    nc = _CACHED["nc"]

    res = run_bass_kernel_spmd(nc, in_maps, core_ids=list(range(N_CORES)))
    return _gather_output(res)
